# revision 18
# baseline (speedup 1.0000x reference)
"""Multi-head attention kernel for 8 TRN2 NeuronCores.

Reference: out = einsum('dha,blha->bld', O, softmax(q k^T) v) with
q/k/v = einsum('dha,bld->blha', W, x);  B=4, L=2048, D=1024, H=16, A=64.

Sharding: core c handles batch b = c//2 and head-group hg = c%2 (8 heads,
data parallel on B x tensor parallel on heads). Each core computes a partial
output [L, D] summed over its 8 heads; the host adds the two head-group
partials per batch.

Design (v3, ACT/PE co-scheduled):
  256 iterations over (strip s in 4 x pair p in 4 x lk-chunk c in 16) with
  512-wide lq strips. Per iteration: 2 score matmuls (K=64, one per head)
  fill one [128,1024] PSUM tile; ONE 1024-wide exp on ACT covers both
  heads; 2 ctx matmuls (K=128, M=65, ones row = free softmax denominators)
  accumulate into per-head [65,512] PSUM tiles. Scores run one iteration
  ahead so ACT (the ~285us exp stream) is the inner-loop pacer.

  x stays resident in SBUF (fp16, one [128,DC,L] tile per tensor, DMAed in
  512-column chunks so the first strip's k/q land ~15us in). Projections
  are "weave units" (k/q: [128,512] W^T@x; v: natural-layout x^T@Vw per
  lk-chunk per pair-half with built-in ones columns) dispatched by a
  deadline queue into per-iteration PE slack. Output projection for strip
  s runs as 4-matmul PSUM bursts woven after strip s completes.

  PSUM: 2x[128,1024] scores + 2x[65,512] ctx + 2x[128,512] utility = 8 banks.

  Normalize per (strip,pair): ctx evac to SBUF bf16, denominator rows
  joined on partition 64, reciprocal_approx_fast in place, then a direct
  partition-broadcast SBUF->SBUF DMA (0-stride source) issued from the
  idle GpSimd queue; odd head enters the pair tile via SBUF->SBUF DMA.

Measured on TRN2 (neuron-profile): see test.py. rel err ~8e-3.
"""

import sys

sys.path.insert(0, "/opt/trn_rl_repo")

from contextlib import ExitStack

import numpy as np
import ml_dtypes

import concourse.bass as bass  # noqa: F401
import concourse.tile as tile
from concourse import bacc, mybir
from concourse.bass_utils import run_bass_kernel_spmd

B, L, D, H, A = 4, 2048, 1024, 16, 64
HC = 8          # heads per core
NP = HC // 2    # head pairs per core
DC = D // 128   # d chunks
LC = L // 128   # lk chunks
HA = HC * A     # 512
SW = 512        # lq strip width
NS = L // SW    # 4 strips
VW = 65         # v block width per head in vnat (64 v + 1 ones)

f32 = mybir.dt.float32
bf16 = mybir.dt.bfloat16
f16 = mybir.dt.float16
ExpF = mybir.ActivationFunctionType.Exp


def build_graph():
    nc = bacc.Bacc("TRN2", target_bir_lowering=False, debug=False, num_devices=8)
    xqT_e = nc.dram_tensor("xqT", [D, L], f16, kind="ExternalInput").ap()
    xkvT_e = nc.dram_tensor("xkvT", [D, L], f16, kind="ExternalInput").ap()
    Qw_e = nc.dram_tensor("Qw", [D, HA], f16, kind="ExternalInput").ap()
    Kw_e = nc.dram_tensor("Kw", [D, HA], f16, kind="ExternalInput").ap()
    Vw_e = nc.dram_tensor("Vw", [D, HA], f16, kind="ExternalInput").ap()
    OwT_e = nc.dram_tensor("OwT", [HA, D], bf16, kind="ExternalInput").ap()
    out_e = nc.dram_tensor("out", [L, D], f32, kind="ExternalOutput").ap()

    with tile.TileContext(nc) as tc, ExitStack() as ctx:
        pers = ctx.enter_context(tc.tile_pool(name="pers", bufs=1))
        psp = ctx.enter_context(tc.tile_pool(name="psp", bufs=1, space="PSUM"))
        etp = ctx.enter_context(tc.tile_pool(name="etp", bufs=3))
        ctpp = ctx.enter_context(tc.tile_pool(name="ctpp", bufs=12))
        nrm = ctx.enter_context(tc.tile_pool(name="nrm", bufs=1))
        unp = ctx.enter_context(tc.tile_pool(name="unp", bufs=2))
        pbp = ctx.enter_context(tc.tile_pool(name="pbp", bufs=2))
        ctop = ctx.enter_context(tc.tile_pool(name="ctop", bufs=2))
        ostp = ctx.enter_context(tc.tile_pool(name="ostp", bufs=3))
        drp = ctx.enter_context(tc.tile_pool(name="drp", bufs=2, space="DRAM"))

        # ---- persistent SBUF ----
        xkv_t = pers.tile([128, DC, L], f16, tag="xkv", name="xkv")
        xq_t = pers.tile([128, DC, L], f16, tag="xq", name="xq")
        kT = [pers.tile([128, L], bf16, tag=f"kT{p}", name=f"kT{p}") for p in range(NP)]
        qT = [pers.tile([128, L], bf16, tag=f"qT{p}", name=f"qT{p}") for p in range(NP)]
        # vnat: [lk-part, lk-chunk, head-blocks of (64 v | 1 ones)]
        vnat = pers.tile([128, LC, HC * VW], bf16, tag="vnat", name="vnat")
        Kw_t = pers.tile([128, DC, HA], f16, tag="Kw", name="Kw")
        Qw_t = pers.tile([128, DC, HA], f16, tag="Qw", name="Qw")
        Vw_t = pers.tile([128, DC, HA], f16, tag="Vw", name="Vw")
        ow = [pers.tile([128, D], bf16, tag=f"ow{c}", name=f"ow{c}") for c in range(NP)]
        warm = pers.tile([1, 16], f32, tag="warm", name="warm")

        # ---- PSUM (exactly 8 banks) ----
        sts = [psp.tile([128, 1024], f32, tag=f"sts{i}", name=f"sts{i}") for i in range(2)]
        pcs = [psp.tile([65, 512], f32, tag=f"pcs{h}", name=f"pcs{h}") for h in range(2)]
        util = [psp.tile([128, 512], f32, tag=f"util{i}", name=f"util{i}") for i in range(2)]

        # ones columns of vnat: element h*65+64 of each (c, h) block
        v0 = vnat[:]
        ones_ap = bass.AP(
            tensor=v0.tensor,
            offset=v0.offset + 64,
            ap=[list(v0.ap[0]), [HC * VW, LC], [VW, HC]],
        )
        nc.vector.memset(ones_ap, 1.0)
        # warm the exp table during idle lead-in
        nc.vector.memset(warm[:], 0.0)
        nc.scalar.activation(warm[:], warm[:], ExpF)

        # ---- DMAs: column-chunked x, ordered for earliest first scores ----
        def w_ap(w_e):
            return bass.AP(
                tensor=w_e.tensor,
                offset=w_e.offset,
                ap=[[HA, 128], [128 * HA, DC], [1, HA]],
            )

        def x_cc_ap(x_e, cc):
            # [128 part, DC d-chunks, 512 cols] of x^T column-chunk cc
            return bass.AP(
                tensor=x_e.tensor,
                offset=x_e.offset + cc * 512,
                ap=[[L, 128], [128 * L, DC], [1, 512]],
            )

        nc.sync.dma_start(out=Kw_t[:], in_=w_ap(Kw_e))
        nc.sync.dma_start(out=xkv_t[:, :, 0:512], in_=x_cc_ap(xkvT_e, 0))
        nc.sync.dma_start(out=Qw_t[:], in_=w_ap(Qw_e))
        nc.sync.dma_start(out=xq_t[:, :, 0:512], in_=x_cc_ap(xqT_e, 0))
        nc.sync.dma_start(out=Vw_t[:], in_=w_ap(Vw_e))
        for cc in range(1, 4):
            nc.sync.dma_start(
                out=xkv_t[:, :, cc * 512:(cc + 1) * 512], in_=x_cc_ap(xkvT_e, cc))
        for cc in range(1, 4):
            nc.sync.dma_start(
                out=xq_t[:, :, cc * 512:(cc + 1) * 512], in_=x_cc_ap(xqT_e, cc))
        for c in range(NP):
            nc.sync.dma_start(out=ow[c][:], in_=OwT_e[c * 128:(c + 1) * 128, :])

        # ---- weave units ----
        util_i = [0]

        def next_util():
            u = util[util_i[0]]
            util_i[0] ^= 1
            return u

        def k_unit(p, lqt):
            u = next_util()
            for d in range(DC):
                nc.tensor.matmul(
                    u[:], lhsT=Kw_t[:, d, p * 128:(p + 1) * 128],
                    rhs=xkv_t[:, d, lqt * 512:(lqt + 1) * 512],
                    start=(d == 0), stop=(d == DC - 1))
            nc.vector.tensor_copy(kT[p][:, lqt * 512:(lqt + 1) * 512], u[:])

        def q_unit(p, s):
            u = next_util()
            for d in range(DC):
                nc.tensor.matmul(
                    u[:], lhsT=Qw_t[:, d, p * 128:(p + 1) * 128],
                    rhs=xq_t[:, d, s * 512:(s + 1) * 512],
                    start=(d == 0), stop=(d == DC - 1))
            nc.vector.tensor_copy(qT[p][:, s * 512:(s + 1) * 512], u[:])

        def v_unit(c, half):
            # natural-layout v for lk-chunk c, heads 4*half..4*half+3
            u = next_util()
            for d in range(DC):
                nc.tensor.matmul(
                    u[:, 0:256], lhsT=xkv_t[:, d, c * 128:(c + 1) * 128],
                    rhs=Vw_t[:, d, half * 256:(half + 1) * 256],
                    start=(d == 0), stop=(d == DC - 1))
            vc = vnat[:, c, :]
            dst = bass.AP(
                tensor=vc.tensor,
                offset=vc.offset + 4 * half * VW,
                ap=[list(vc.ap[0]), [VW, 4], [1, 64]],
            )
            ua = u[:]
            src = bass.AP(
                tensor=ua.tensor,
                offset=ua.offset,
                ap=[list(ua.ap[0]), [64, 4], [1, 64]],
            )
            nc.vector.tensor_copy(dst, src)

        ctp = {}

        def opj_unit(s, lqs, dt):
            u = next_util()
            for p in range(NP):
                nc.tensor.matmul(
                    u[:], lhsT=ctp[(s, p)][:, lqs * 128:(lqs + 1) * 128],
                    rhs=ow[p][:, dt * 512:(dt + 1) * 512],
                    start=(p == 0), stop=(p == NP - 1))
            row = s * 512 + lqs * 128
            ost = ostp.tile([128, 512], f32, tag="ost", name=f"ost{s}_{lqs}_{dt}")
            nc.vector.tensor_copy(ost[:], u[:])
            nc.sync.dma_start(
                out=out_e[row:row + 128, dt * 512:(dt + 1) * 512], in_=ost[:])

        # ---- deadline-queue weave schedule ----
        # (deadline_iter, min_iter, unit); popped when deadline <= it+3, or
        # one filler per iteration once min_iter is reached.
        wq = []
        for lqt in range(1, 4):
            wq.append((4 * lqt - 1, 0, lambda lqt=lqt: k_unit(0, lqt)))
        for p in range(1, NP):
            for lqt in range(4):
                wq.append((16 * p + 4 * lqt - 1, 0, lambda p=p, lqt=lqt: k_unit(p, lqt)))
        for p in range(NP):
            for s in range(NS):
                if p == 0 and s == 0:
                    continue  # in lead
                wq.append((16 * (4 * s + p) - 1, 0, lambda p=p, s=s: q_unit(p, s)))
        for c in range(1, LC):
            wq.append((c, 0, lambda c=c: v_unit(c, 0)))
        for c in range(LC):
            wq.append((32 + c, 12, lambda c=c: v_unit(c, 1)))
        for s in range(NS - 1):
            for j, (lqs, dt) in enumerate((a, b) for a in range(4) for b in range(2)):
                wq.append((16 * (4 * s + 4) + 6 + 2 * j, 16 * (4 * s + 4) + 4,
                           lambda s=s, lqs=lqs, dt=dt: opj_unit(s, lqs, dt)))
        wq.sort(key=lambda t: t[0])

        # ---- normalize + pair-tile assembly per round ----
        def finalize(s, p):
            un = []
            for h2 in range(2):
                ut = unp.tile([64, 512], bf16, tag="un", name=f"un{s}_{p}_{h2}")
                nc.vector.tensor_copy(ut[:], pcs[h2][0:64, :])
                un.append(ut)
            stage = nrm.tile([65, 1024], f32, tag="stage", name=f"stage{s}_{p}")
            for h2 in range(2):
                nc.vector.tensor_copy(
                    stage[64:65, h2 * 512:(h2 + 1) * 512], pcs[h2][64:65, :])
            den = nrm.tile([2, 512], f32, tag="den", name=f"den{s}_{p}")
            nc.sync.dma_start(out=den[:], in_=stage[64:65, :])
            rec = nrm.tile([2, 512], f32, tag="rec", name=f"rec{s}_{p}")
            nc.vector.reciprocal_approx_fast(rec[:], den[:])
            # partition broadcast via DRAM bounce (0-stride partition reads
            # are only legal on DRAM APs)
            dr = drp.tile([2, 512], f32, tag="dr", name=f"dr{s}_{p}")
            nc.sync.dma_start(out=dr[:], in_=rec[:])
            ct = ctpp.tile([128, 512], bf16, tag="ctp", name=f"ctp{s}_{p}")
            pbs = []
            for h2 in range(2):
                pb = pbp.tile([64, 512], f32, tag="pbs", name=f"pbs{s}_{p}_{h2}")
                rrow = dr[h2:h2 + 1, :]
                rbc = bass.AP(
                    tensor=rrow.tensor, offset=rrow.offset,
                    ap=[[0, 64], [1, 512]])
                nc.sync.dma_start(out=pb[:], in_=rbc)
                pbs.append(pb)
            # odd head first: its SBUF->SBUF placement DMA overlaps h0's mult
            cto = ctop.tile([64, 512], bf16, tag="cto", name=f"cto{s}_{p}")
            nc.vector.tensor_mul(cto[:], un[1][:], pbs[1][:])
            nc.sync.dma_start(out=ct[64:128, :], in_=cto[:])
            nc.vector.tensor_mul(ct[0:64, :], un[0][:], pbs[0][:])
            ctp[(s, p)] = ct
            return {"un": un, "stage": stage, "den": den, "rec": rec,
                    "pbs": pbs, "cto": cto}

        # ---- main pipeline ----
        sched = [(s, p, c) for s in range(NS) for p in range(NP) for c in range(LC)]
        NIT = len(sched)

        def emit_scores(it):
            s, p, c = sched[it]
            st = sts[it % 2]
            for h2 in range(2):
                base = 64 * h2
                nc.tensor.matmul(
                    st[:, h2 * 512:(h2 + 1) * 512],
                    lhsT=kT[p][base:base + 64, c * 128:(c + 1) * 128],
                    rhs=qT[p][base:base + 64, s * 512:(s + 1) * 512],
                    start=True, stop=True)

        # lead: minimal critical path to the first exp
        k_unit(0, 0)
        q_unit(0, 0)
        emit_scores(0)
        v_unit(0, 0)

        for it, (s, p, c) in enumerate(sched):
            if it + 1 < NIT:
                emit_scores(it + 1)
            et = etp.tile([128, 1024], bf16, tag="et", name=f"et{it}")
            nc.scalar.activation(et[:], sts[it % 2][:], ExpF)
            for h2 in range(2):
                nc.tensor.matmul(
                    pcs[h2][:],
                    lhsT=vnat[:, c, (2 * p + h2) * VW:(2 * p + h2) * VW + VW],
                    rhs=et[:, h2 * 512:(h2 + 1) * 512],
                    start=(c == 0), stop=(c == LC - 1))
            emitted = 0
            while wq and wq[0][0] <= it + 3 and emitted < 2:
                wq.pop(0)[2]()
                emitted += 1
            if not emitted and wq and wq[0][1] <= it:
                wq.pop(0)[2]()
            if c == LC - 1:
                finalize(s, p)

        while wq:
            wq.pop(0)[2]()

        # ---- tail: last strip's output projection ----
        for lqs in range(4):
            for dt in range(2):
                opj_unit(NS - 1, lqs, dt)

    nc.compile()
    return nc


_NC = None


def _get_nc():
    global _NC
    if _NC is None:
        _NC = build_graph()
    return _NC


# test harness can override, e.g. {"trace": True}
RUN_KWARGS: dict = {}
LAST_RESULTS = None


def make_in_maps(xq, xkv, Q, K, V, O):
    xq = np.asarray(xq, np.float32)
    xkv = np.asarray(xkv, np.float32)
    Q = np.asarray(Q, np.float32)
    K = np.asarray(K, np.float32)
    V = np.asarray(V, np.float32)
    O = np.asarray(O, np.float32)
    # cores 2b and 2b+1 share batch b's transposed activations; compute once
    xqT_c = [np.ascontiguousarray(xq[b].T).astype(np.float16) for b in range(B)]
    xkvT_c = [np.ascontiguousarray(xkv[b].T).astype(np.float16) for b in range(B)]
    in_maps = []
    for core in range(8):
        b, hg = divmod(core, 2)
        hs = slice(hg * HC, (hg + 1) * HC)
        in_maps.append({
            "xqT": xqT_c[b],
            "xkvT": xkvT_c[b],
            "Qw": np.ascontiguousarray(Q[:, hs, :].reshape(D, HA)).astype(np.float16),
            "Kw": np.ascontiguousarray(K[:, hs, :].reshape(D, HA)).astype(np.float16),
            "Vw": np.ascontiguousarray(V[:, hs, :].reshape(D, HA)).astype(np.float16),
            "OwT": np.ascontiguousarray(
                O[:, hs, :].reshape(D, HA).T).astype(ml_dtypes.bfloat16),
        })
    return in_maps


def kernel(xq, xkv, Q, K, V, O):
    global LAST_RESULTS
    nc = _get_nc()
    in_maps = make_in_maps(xq, xkv, Q, K, V, O)
    res = run_bass_kernel_spmd(nc, in_maps, core_ids=list(range(8)), **RUN_KWARGS)
    LAST_RESULTS = res
    outs = [np.asarray(res.results[c]["out"], np.float32) for c in range(8)]
    return np.stack([outs[2 * b] + outs[2 * b + 1] for b in range(B)], axis=0)


# revision 22
# speedup vs baseline: 1.1737x; 1.1737x over previous
"""Multi-head attention kernel for 8 TRN2 NeuronCores.

Reference: out = einsum('dha,blha->bld', O, softmax(q k^T) v) with
q/k/v = einsum('dha,bld->blha', W, x);  B=4, L=2048, D=1024, H=16, A=64.

Sharding: core c handles batch b = c//2 and head-group hg = c%2 (8 heads,
data parallel on B x tensor parallel on heads). Each core computes a partial
output [L, D] summed over its 8 heads; the host adds the two head-group
partials per batch.

Design (v3, ACT/PE co-scheduled):
  256 iterations over (strip s in 4 x pair p in 4 x lk-chunk c in 16) with
  512-wide lq strips. Per iteration: 2 score matmuls (K=64, one per head)
  fill one [128,1024] PSUM tile; ONE 1024-wide exp on ACT covers both
  heads; 2 ctx matmuls (K=128, M=65, ones row = free softmax denominators)
  accumulate into per-head [65,512] PSUM tiles. Scores run one iteration
  ahead so ACT (the ~285us exp stream) is the inner-loop pacer.

  x stays resident in SBUF (fp16, one [128,DC,L] tile per tensor, DMAed in
  512-column chunks so the first strip's k/q land ~15us in). Projections
  are "weave units" (k/q: [128,512] W^T@x; v: natural-layout x^T@Vw per
  lk-chunk per pair-half with built-in ones columns) dispatched by a
  deadline queue into per-iteration PE slack. Output projection for strip
  s runs as 4-matmul PSUM bursts woven after strip s completes.

  PSUM: 2x[128,1024] scores + 2x[65,512] ctx + 2x[128,512] utility = 8 banks.

  Normalize per (strip,pair): ctx evac to SBUF bf16, denominator rows
  joined on partition 64, reciprocal_approx_fast in place, then a direct
  partition-broadcast SBUF->SBUF DMA (0-stride source) issued from the
  idle GpSimd queue; odd head enters the pair tile via SBUF->SBUF DMA.

Measured on TRN2 (neuron-profile): see test.py. rel err ~8e-3.
"""

import sys

sys.path.insert(0, "/opt/trn_rl_repo")

from contextlib import ExitStack

import numpy as np
import ml_dtypes

import concourse.bass as bass  # noqa: F401
import concourse.tile as tile
from concourse import bacc, mybir
from concourse.bass_utils import run_bass_kernel_spmd

B, L, D, H, A = 4, 2048, 1024, 16, 64
HC = 8          # heads per core
NP = HC // 2    # head pairs per core
DC = D // 128   # d chunks
LC = L // 128   # lk chunks
HA = HC * A     # 512
SW = 512        # lq strip width
NS = L // SW    # 4 strips
VW = 65         # v block width per head in vnat (64 v + 1 ones)

f32 = mybir.dt.float32
bf16 = mybir.dt.bfloat16
f16 = mybir.dt.float16
ExpF = mybir.ActivationFunctionType.Exp


def build_graph():
    nc = bacc.Bacc("TRN2", target_bir_lowering=False, debug=False, num_devices=8)
    xqT_e = nc.dram_tensor("xqT", [D, L], f16, kind="ExternalInput").ap()
    xkvT_e = nc.dram_tensor("xkvT", [D, L], f16, kind="ExternalInput").ap()
    Qw_e = nc.dram_tensor("Qw", [D, HA], f16, kind="ExternalInput").ap()
    Kw_e = nc.dram_tensor("Kw", [D, HA], f16, kind="ExternalInput").ap()
    Vw_e = nc.dram_tensor("Vw", [D, HA], f16, kind="ExternalInput").ap()
    OwT_e = nc.dram_tensor("OwT", [HA, D], bf16, kind="ExternalInput").ap()
    out_e = nc.dram_tensor("out", [L, D], f32, kind="ExternalOutput").ap()

    with tile.TileContext(nc) as tc, ExitStack() as ctx:
        pers = ctx.enter_context(tc.tile_pool(name="pers", bufs=1))
        psp = ctx.enter_context(tc.tile_pool(name="psp", bufs=1, space="PSUM"))
        etp = ctx.enter_context(tc.tile_pool(name="etp", bufs=3))
        ctpp = ctx.enter_context(tc.tile_pool(name="ctpp", bufs=12))
        nrm = ctx.enter_context(tc.tile_pool(name="nrm", bufs=1))
        unp = ctx.enter_context(tc.tile_pool(name="unp", bufs=2))
        pbp = ctx.enter_context(tc.tile_pool(name="pbp", bufs=2))
        ctop = ctx.enter_context(tc.tile_pool(name="ctop", bufs=2))
        ostp = ctx.enter_context(tc.tile_pool(name="ostp", bufs=3))
        drp = ctx.enter_context(tc.tile_pool(name="drp", bufs=2, space="DRAM"))

        # ---- persistent SBUF ----
        xkv_t = pers.tile([128, DC, L], f16, tag="xkv", name="xkv")
        xq_t = pers.tile([128, DC, L], f16, tag="xq", name="xq")
        kT = [pers.tile([128, L], bf16, tag=f"kT{p}", name=f"kT{p}") for p in range(NP)]
        qT = [pers.tile([128, L], bf16, tag=f"qT{p}", name=f"qT{p}") for p in range(NP)]
        # vnat: [lk-part, lk-chunk, head-blocks of (64 v | 1 ones)]
        vnat = pers.tile([128, LC, HC * VW], bf16, tag="vnat", name="vnat")
        Kw_t = pers.tile([128, DC, HA], f16, tag="Kw", name="Kw")
        Qw_t = pers.tile([128, DC, HA], f16, tag="Qw", name="Qw")
        Vw_t = pers.tile([128, DC, HA], f16, tag="Vw", name="Vw")
        ow = [pers.tile([128, D], bf16, tag=f"ow{c}", name=f"ow{c}") for c in range(NP)]
        warm = pers.tile([1, 16], f32, tag="warm", name="warm")

        # ---- PSUM (exactly 8 banks) ----
        sts = [psp.tile([128, 1024], f32, tag=f"sts{i}", name=f"sts{i}") for i in range(2)]
        pcs = [psp.tile([65, 512], f32, tag=f"pcs{h}", name=f"pcs{h}") for h in range(2)]
        util = [psp.tile([128, 512], f32, tag=f"util{i}", name=f"util{i}") for i in range(2)]

        # ones columns of vnat: element h*65+64 of each (c, h) block
        v0 = vnat[:]
        ones_ap = bass.AP(
            tensor=v0.tensor,
            offset=v0.offset + 64,
            ap=[list(v0.ap[0]), [HC * VW, LC], [VW, HC]],
        )
        nc.vector.memset(ones_ap, 1.0)
        # warm the exp table during idle lead-in
        nc.vector.memset(warm[:], 0.0)
        nc.scalar.activation(warm[:], warm[:], ExpF)

        # ---- DMAs: column-chunked x, ordered for earliest first scores ----
        def w_ap(w_e):
            return bass.AP(
                tensor=w_e.tensor,
                offset=w_e.offset,
                ap=[[HA, 128], [128 * HA, DC], [1, HA]],
            )

        def x_cc_ap(x_e, cc):
            # [128 part, DC d-chunks, 512 cols] of x^T column-chunk cc
            return bass.AP(
                tensor=x_e.tensor,
                offset=x_e.offset + cc * 512,
                ap=[[L, 128], [128 * L, DC], [1, 512]],
            )

        nc.sync.dma_start(out=Kw_t[:], in_=w_ap(Kw_e))
        nc.sync.dma_start(out=xkv_t[:, :, 0:512], in_=x_cc_ap(xkvT_e, 0))
        nc.sync.dma_start(out=Qw_t[:], in_=w_ap(Qw_e))
        nc.sync.dma_start(out=xq_t[:, :, 0:512], in_=x_cc_ap(xqT_e, 0))
        nc.sync.dma_start(out=Vw_t[:], in_=w_ap(Vw_e))
        for cc in range(1, 4):
            nc.sync.dma_start(
                out=xkv_t[:, :, cc * 512:(cc + 1) * 512], in_=x_cc_ap(xkvT_e, cc))
        for cc in range(1, 4):
            nc.sync.dma_start(
                out=xq_t[:, :, cc * 512:(cc + 1) * 512], in_=x_cc_ap(xqT_e, cc))
        for c in range(NP):
            nc.sync.dma_start(out=ow[c][:], in_=OwT_e[c * 128:(c + 1) * 128, :])

        # ---- weave units ----
        util_i = [0]

        def next_util():
            u = util[util_i[0]]
            util_i[0] ^= 1
            return u

        def k_unit(p, lqt):
            u = next_util()
            for d in range(DC):
                nc.tensor.matmul(
                    u[:], lhsT=Kw_t[:, d, p * 128:(p + 1) * 128],
                    rhs=xkv_t[:, d, lqt * 512:(lqt + 1) * 512],
                    start=(d == 0), stop=(d == DC - 1))
            nc.vector.tensor_copy(kT[p][:, lqt * 512:(lqt + 1) * 512], u[:])

        def q_unit(p, s):
            u = next_util()
            for d in range(DC):
                nc.tensor.matmul(
                    u[:], lhsT=Qw_t[:, d, p * 128:(p + 1) * 128],
                    rhs=xq_t[:, d, s * 512:(s + 1) * 512],
                    start=(d == 0), stop=(d == DC - 1))
            nc.vector.tensor_copy(qT[p][:, s * 512:(s + 1) * 512], u[:])

        def v_unit(c, half):
            # natural-layout v for lk-chunk c, heads 4*half..4*half+3
            u = next_util()
            for d in range(DC):
                nc.tensor.matmul(
                    u[:, 0:256], lhsT=xkv_t[:, d, c * 128:(c + 1) * 128],
                    rhs=Vw_t[:, d, half * 256:(half + 1) * 256],
                    start=(d == 0), stop=(d == DC - 1))
            vc = vnat[:, c, :]
            dst = bass.AP(
                tensor=vc.tensor,
                offset=vc.offset + 4 * half * VW,
                ap=[list(vc.ap[0]), [VW, 4], [1, 64]],
            )
            ua = u[:]
            src = bass.AP(
                tensor=ua.tensor,
                offset=ua.offset,
                ap=[list(ua.ap[0]), [64, 4], [1, 64]],
            )
            nc.vector.tensor_copy(dst, src)

        ctp = {}

        def opj_unit(s, lqs, dt):
            u = next_util()
            for p in range(NP):
                nc.tensor.matmul(
                    u[:], lhsT=ctp[(s, p)][:, lqs * 128:(lqs + 1) * 128],
                    rhs=ow[p][:, dt * 512:(dt + 1) * 512],
                    start=(p == 0), stop=(p == NP - 1))
            row = s * 512 + lqs * 128
            ost = ostp.tile([128, 512], f32, tag="ost", name=f"ost{s}_{lqs}_{dt}")
            nc.vector.tensor_copy(ost[:], u[:])
            nc.sync.dma_start(
                out=out_e[row:row + 128, dt * 512:(dt + 1) * 512], in_=ost[:])

        # ---- deadline-queue weave schedule ----
        # (deadline_iter, min_iter, unit); popped when deadline <= it+3, or
        # one filler per iteration once min_iter is reached.
        wq = []
        for lqt in range(1, 4):
            wq.append((4 * lqt - 1, 0, lambda lqt=lqt: k_unit(0, lqt)))
        for p in range(1, NP):
            for lqt in range(4):
                wq.append((16 * p + 4 * lqt - 1, 0, lambda p=p, lqt=lqt: k_unit(p, lqt)))
        for p in range(NP):
            for s in range(NS):
                if p == 0 and s == 0:
                    continue  # in lead
                wq.append((16 * (4 * s + p) - 1, 0, lambda p=p, s=s: q_unit(p, s)))
        for c in range(1, LC):
            wq.append((c, 0, lambda c=c: v_unit(c, 0)))
        for c in range(LC):
            wq.append((32 + c, 12, lambda c=c: v_unit(c, 1)))
        for s in range(NS - 1):
            for j, (lqs, dt) in enumerate((a, b) for a in range(4) for b in range(2)):
                wq.append((16 * (4 * s + 4) + 6 + 2 * j, 16 * (4 * s + 4) + 4,
                           lambda s=s, lqs=lqs, dt=dt: opj_unit(s, lqs, dt)))
        wq.sort(key=lambda t: t[0])

        # ---- normalize + pair-tile assembly per round ----
        def finalize(s, p, direct=False):
            # stage copies first: they feed the long denominator DMA chain
            stage = nrm.tile([65, 1024], f32, tag="stage", name=f"stage{s}_{p}")
            for h2 in range(2):
                nc.vector.tensor_copy(
                    stage[64:65, h2 * 512:(h2 + 1) * 512], pcs[h2][64:65, :])
            if direct:
                # last round: no next round needs pcs, multiply straight from
                # PSUM and skip the evacuation hop
                un = [pcs[0][0:64, :], pcs[1][0:64, :]]
            else:
                un = []
                for h2 in range(2):
                    ut = unp.tile([64, 512], bf16, tag="un", name=f"un{s}_{p}_{h2}")
                    nc.vector.tensor_copy(ut[:], pcs[h2][0:64, :])
                    un.append(ut[:])
            den = nrm.tile([2, 512], f32, tag="den", name=f"den{s}_{p}")
            nc.sync.dma_start(out=den[:], in_=stage[64:65, :])
            rec = nrm.tile([2, 512], f32, tag="rec", name=f"rec{s}_{p}")
            nc.vector.reciprocal_approx_fast(rec[:], den[:])
            # partition broadcast via DRAM bounce (0-stride partition reads
            # are only legal on DRAM APs)
            dr = drp.tile([2, 512], f32, tag="dr", name=f"dr{s}_{p}")
            nc.sync.dma_start(out=dr[:], in_=rec[:])
            ct = ctpp.tile([128, 512], bf16, tag="ctp", name=f"ctp{s}_{p}")
            pbs = []
            for h2 in range(2):
                pb = pbp.tile([64, 512], f32, tag="pbs", name=f"pbs{s}_{p}_{h2}")
                rrow = dr[h2:h2 + 1, :]
                rbc = bass.AP(
                    tensor=rrow.tensor, offset=rrow.offset,
                    ap=[[0, 64], [1, 512]])
                nc.sync.dma_start(out=pb[:], in_=rbc)
                pbs.append(pb)
            # odd head first: its SBUF->SBUF placement DMA overlaps h0's mult
            cto = ctop.tile([64, 512], bf16, tag="cto", name=f"cto{s}_{p}")
            nc.vector.tensor_mul(cto[:], un[1], pbs[1][:])
            nc.sync.dma_start(out=ct[64:128, :], in_=cto[:])
            nc.vector.tensor_mul(ct[0:64, :], un[0], pbs[0][:])
            ctp[(s, p)] = ct

        # ---- main pipeline ----
        sched = [(s, p, c) for s in range(NS) for p in range(NP) for c in range(LC)]
        NIT = len(sched)

        def emit_scores(it):
            s, p, c = sched[it]
            st = sts[it % 2]
            for h2 in range(2):
                base = 64 * h2
                nc.tensor.matmul(
                    st[:, h2 * 512:(h2 + 1) * 512],
                    lhsT=kT[p][base:base + 64, c * 128:(c + 1) * 128],
                    rhs=qT[p][base:base + 64, s * 512:(s + 1) * 512],
                    start=True, stop=True)

        # lead: minimal critical path to the first exp
        k_unit(0, 0)
        q_unit(0, 0)
        emit_scores(0)
        v_unit(0, 0)

        for it, (s, p, c) in enumerate(sched):
            if it + 1 < NIT:
                emit_scores(it + 1)
            et = etp.tile([128, 1024], bf16, tag="et", name=f"et{it}")
            nc.scalar.activation(et[:], sts[it % 2][:], ExpF)
            # weave before ctx: the PE chews projection/outproj work while
            # this iteration's exp finishes, instead of stalling at ctx
            emitted = 0
            while wq and wq[0][0] <= it + 3 and emitted < 2:
                wq.pop(0)[2]()
                emitted += 1
            if not emitted and wq and wq[0][1] <= it:
                wq.pop(0)[2]()
            for h2 in range(2):
                nc.tensor.matmul(
                    pcs[h2][:],
                    lhsT=vnat[:, c, (2 * p + h2) * VW:(2 * p + h2) * VW + VW],
                    rhs=et[:, h2 * 512:(h2 + 1) * 512],
                    start=(c == 0), stop=(c == LC - 1))
            if c == LC - 1:
                finalize(s, p, direct=(it == NIT - 1))

        while wq:
            wq.pop(0)[2]()

        # ---- tail: last strip's output projection ----
        for lqs in range(4):
            for dt in range(2):
                opj_unit(NS - 1, lqs, dt)

    nc.compile()
    return nc


_NC = None


def _get_nc():
    global _NC
    if _NC is None:
        _NC = build_graph()
    return _NC


# test harness can override, e.g. {"trace": True}
RUN_KWARGS: dict = {}
LAST_RESULTS = None


def make_in_maps(xq, xkv, Q, K, V, O):
    xq = np.asarray(xq, np.float32)
    xkv = np.asarray(xkv, np.float32)
    Q = np.asarray(Q, np.float32)
    K = np.asarray(K, np.float32)
    V = np.asarray(V, np.float32)
    O = np.asarray(O, np.float32)
    # cores 2b and 2b+1 share batch b's transposed activations; compute once
    xqT_c = [np.ascontiguousarray(xq[b].T).astype(np.float16) for b in range(B)]
    xkvT_c = [np.ascontiguousarray(xkv[b].T).astype(np.float16) for b in range(B)]
    in_maps = []
    for core in range(8):
        b, hg = divmod(core, 2)
        hs = slice(hg * HC, (hg + 1) * HC)
        in_maps.append({
            "xqT": xqT_c[b],
            "xkvT": xkvT_c[b],
            "Qw": np.ascontiguousarray(Q[:, hs, :].reshape(D, HA)).astype(np.float16),
            "Kw": np.ascontiguousarray(K[:, hs, :].reshape(D, HA)).astype(np.float16),
            "Vw": np.ascontiguousarray(V[:, hs, :].reshape(D, HA)).astype(np.float16),
            "OwT": np.ascontiguousarray(
                O[:, hs, :].reshape(D, HA).T).astype(ml_dtypes.bfloat16),
        })
    return in_maps


def kernel(xq, xkv, Q, K, V, O):
    global LAST_RESULTS
    nc = _get_nc()
    in_maps = make_in_maps(xq, xkv, Q, K, V, O)
    res = run_bass_kernel_spmd(nc, in_maps, core_ids=list(range(8)), **RUN_KWARGS)
    LAST_RESULTS = res
    outs = [np.asarray(res.results[c]["out"], np.float32) for c in range(8)]
    return np.stack([outs[2 * b] + outs[2 * b + 1] for b in range(B)], axis=0)


# revision 24
# speedup vs baseline: 1.2042x; 1.0260x over previous
"""Multi-head attention kernel for 8 TRN2 NeuronCores.

Reference: out = einsum('dha,blha->bld', O, softmax(q k^T) v) with
q/k/v = einsum('dha,bld->blha', W, x);  B=4, L=2048, D=1024, H=16, A=64.

Sharding: core c handles batch b = c//2 and head-group hg = c%2 (8 heads,
data parallel on B x tensor parallel on heads). Each core computes a partial
output [L, D] summed over its 8 heads; the host adds the two head-group
partials per batch.

Design (v3, ACT/PE co-scheduled):
  256 iterations over (strip s in 4 x pair p in 4 x lk-chunk c in 16) with
  512-wide lq strips. Per iteration: 2 score matmuls (K=64, one per head)
  fill one [128,1024] PSUM tile; ONE 1024-wide exp on ACT covers both
  heads; 2 ctx matmuls (K=128, M=65, ones row = free softmax denominators)
  accumulate into per-head [65,512] PSUM tiles. Scores run one iteration
  ahead so ACT (the ~285us exp stream) is the inner-loop pacer.

  x stays resident in SBUF (fp16, one [128,DC,L] tile per tensor, DMAed in
  512-column chunks so the first strip's k/q land ~15us in). Projections
  are "weave units" (k/q: [128,512] W^T@x; v: natural-layout x^T@Vw per
  lk-chunk per pair-half with built-in ones columns) dispatched by a
  deadline queue into per-iteration PE slack. Output projection for strip
  s runs as 4-matmul PSUM bursts woven after strip s completes.

  PSUM: 2x[128,1024] scores + 2x[65,512] ctx + 2x[128,512] utility = 8 banks.

  Normalize per (strip,pair): ctx evac to SBUF bf16, denominator rows
  joined on partition 64, reciprocal_approx_fast in place, then a direct
  partition-broadcast SBUF->SBUF DMA (0-stride source) issued from the
  idle GpSimd queue; odd head enters the pair tile via SBUF->SBUF DMA.

Measured on TRN2 (neuron-profile): see test.py. rel err ~8e-3.
"""

import sys

sys.path.insert(0, "/opt/trn_rl_repo")

from contextlib import ExitStack

import numpy as np
import ml_dtypes

import concourse.bass as bass  # noqa: F401
import concourse.tile as tile
from concourse import bacc, mybir
from concourse.bass_utils import run_bass_kernel_spmd

B, L, D, H, A = 4, 2048, 1024, 16, 64
HC = 8          # heads per core
NP = HC // 2    # head pairs per core
DC = D // 128   # d chunks
LC = L // 128   # lk chunks
HA = HC * A     # 512
SW = 512        # lq strip width
NS = L // SW    # 4 strips
VW = 65         # v block width per head in vnat (64 v + 1 ones)

f32 = mybir.dt.float32
bf16 = mybir.dt.bfloat16
f16 = mybir.dt.float16
ExpF = mybir.ActivationFunctionType.Exp


def build_graph():
    nc = bacc.Bacc("TRN2", target_bir_lowering=False, debug=False, num_devices=8)
    xqT_e = nc.dram_tensor("xqT", [D, L], f16, kind="ExternalInput").ap()
    xkvT_e = nc.dram_tensor("xkvT", [D, L], f16, kind="ExternalInput").ap()
    Qw_e = nc.dram_tensor("Qw", [D, HA], f16, kind="ExternalInput").ap()
    Kw_e = nc.dram_tensor("Kw", [D, HA], f16, kind="ExternalInput").ap()
    Vw_e = nc.dram_tensor("Vw", [D, HA], f16, kind="ExternalInput").ap()
    OwT_e = nc.dram_tensor("OwT", [HA, D], bf16, kind="ExternalInput").ap()
    out_e = nc.dram_tensor("out", [L, D], f32, kind="ExternalOutput").ap()

    with tile.TileContext(nc) as tc, ExitStack() as ctx:
        pers = ctx.enter_context(tc.tile_pool(name="pers", bufs=1))
        psp = ctx.enter_context(tc.tile_pool(name="psp", bufs=1, space="PSUM"))
        etp = ctx.enter_context(tc.tile_pool(name="etp", bufs=3))
        ctpp = ctx.enter_context(tc.tile_pool(name="ctpp", bufs=12))
        nrm = ctx.enter_context(tc.tile_pool(name="nrm", bufs=1))
        unp = ctx.enter_context(tc.tile_pool(name="unp", bufs=2))
        pbp = ctx.enter_context(tc.tile_pool(name="pbp", bufs=2))
        ctop = ctx.enter_context(tc.tile_pool(name="ctop", bufs=2))
        ostp = ctx.enter_context(tc.tile_pool(name="ostp", bufs=3))
        drp = ctx.enter_context(tc.tile_pool(name="drp", bufs=2, space="DRAM"))

        # ---- persistent SBUF ----
        xkv_t = pers.tile([128, DC, L], f16, tag="xkv", name="xkv")
        xq_t = pers.tile([128, DC, L], f16, tag="xq", name="xq")
        kT = [pers.tile([128, L], bf16, tag=f"kT{p}", name=f"kT{p}") for p in range(NP)]
        qT = [pers.tile([128, L], bf16, tag=f"qT{p}", name=f"qT{p}") for p in range(NP)]
        # vnat: [lk-part, lk-chunk, head-blocks of (64 v | 1 ones)]
        vnat = pers.tile([128, LC, HC * VW], bf16, tag="vnat", name="vnat")
        Kw_t = pers.tile([128, DC, HA], f16, tag="Kw", name="Kw")
        Qw_t = pers.tile([128, DC, HA], f16, tag="Qw", name="Qw")
        Vw_t = pers.tile([128, DC, HA], f16, tag="Vw", name="Vw")
        ow = [pers.tile([128, D], bf16, tag=f"ow{c}", name=f"ow{c}") for c in range(NP)]
        warm = pers.tile([1, 16], f32, tag="warm", name="warm")
        # ones row on partition 64: stationary for the K=1 broadcast matmul
        ones65 = pers.tile([65, 64], f32, tag="ones65", name="ones65")
        nc.vector.memset(ones65[:], 1.0)

        # ---- PSUM (exactly 8 banks) ----
        sts = [psp.tile([128, 1024], f32, tag=f"sts{i}", name=f"sts{i}") for i in range(2)]
        pcs = [psp.tile([65, 512], f32, tag=f"pcs{h}", name=f"pcs{h}") for h in range(2)]
        util = [psp.tile([128, 512], f32, tag=f"util{i}", name=f"util{i}") for i in range(2)]

        # ones columns of vnat: element h*65+64 of each (c, h) block
        v0 = vnat[:]
        ones_ap = bass.AP(
            tensor=v0.tensor,
            offset=v0.offset + 64,
            ap=[list(v0.ap[0]), [HC * VW, LC], [VW, HC]],
        )
        nc.vector.memset(ones_ap, 1.0)
        # warm the exp table during idle lead-in
        nc.vector.memset(warm[:], 0.0)
        nc.scalar.activation(warm[:], warm[:], ExpF)

        # ---- DMAs: column-chunked x, ordered for earliest first scores ----
        def w_ap(w_e):
            return bass.AP(
                tensor=w_e.tensor,
                offset=w_e.offset,
                ap=[[HA, 128], [128 * HA, DC], [1, HA]],
            )

        def x_cc_ap(x_e, cc):
            # [128 part, DC d-chunks, 512 cols] of x^T column-chunk cc
            return bass.AP(
                tensor=x_e.tensor,
                offset=x_e.offset + cc * 512,
                ap=[[L, 128], [128 * L, DC], [1, 512]],
            )

        nc.sync.dma_start(out=Kw_t[:], in_=w_ap(Kw_e))
        nc.sync.dma_start(out=xkv_t[:, :, 0:512], in_=x_cc_ap(xkvT_e, 0))
        nc.sync.dma_start(out=Qw_t[:], in_=w_ap(Qw_e))
        nc.sync.dma_start(out=xq_t[:, :, 0:512], in_=x_cc_ap(xqT_e, 0))
        nc.sync.dma_start(out=Vw_t[:], in_=w_ap(Vw_e))
        for cc in range(1, 4):
            nc.sync.dma_start(
                out=xkv_t[:, :, cc * 512:(cc + 1) * 512], in_=x_cc_ap(xkvT_e, cc))
        for cc in range(1, 4):
            nc.sync.dma_start(
                out=xq_t[:, :, cc * 512:(cc + 1) * 512], in_=x_cc_ap(xqT_e, cc))
        for c in range(NP):
            nc.sync.dma_start(out=ow[c][:], in_=OwT_e[c * 128:(c + 1) * 128, :])

        # ---- weave units ----
        util_i = [0]

        def next_util():
            u = util[util_i[0]]
            util_i[0] ^= 1
            return u

        def k_unit(p, lqt):
            u = next_util()
            for d in range(DC):
                nc.tensor.matmul(
                    u[:], lhsT=Kw_t[:, d, p * 128:(p + 1) * 128],
                    rhs=xkv_t[:, d, lqt * 512:(lqt + 1) * 512],
                    start=(d == 0), stop=(d == DC - 1))
            nc.vector.tensor_copy(kT[p][:, lqt * 512:(lqt + 1) * 512], u[:])

        def q_unit(p, s):
            u = next_util()
            for d in range(DC):
                nc.tensor.matmul(
                    u[:], lhsT=Qw_t[:, d, p * 128:(p + 1) * 128],
                    rhs=xq_t[:, d, s * 512:(s + 1) * 512],
                    start=(d == 0), stop=(d == DC - 1))
            nc.vector.tensor_copy(qT[p][:, s * 512:(s + 1) * 512], u[:])

        def v_unit(c, half):
            # natural-layout v for lk-chunk c, heads 4*half..4*half+3
            u = next_util()
            for d in range(DC):
                nc.tensor.matmul(
                    u[:, 0:256], lhsT=xkv_t[:, d, c * 128:(c + 1) * 128],
                    rhs=Vw_t[:, d, half * 256:(half + 1) * 256],
                    start=(d == 0), stop=(d == DC - 1))
            vc = vnat[:, c, :]
            dst = bass.AP(
                tensor=vc.tensor,
                offset=vc.offset + 4 * half * VW,
                ap=[list(vc.ap[0]), [VW, 4], [1, 64]],
            )
            ua = u[:]
            src = bass.AP(
                tensor=ua.tensor,
                offset=ua.offset,
                ap=[list(ua.ap[0]), [64, 4], [1, 64]],
            )
            nc.vector.tensor_copy(dst, src)

        ctp = {}

        def opj_unit(s, lqs, dt):
            u = next_util()
            for p in range(NP):
                nc.tensor.matmul(
                    u[:], lhsT=ctp[(s, p)][:, lqs * 128:(lqs + 1) * 128],
                    rhs=ow[p][:, dt * 512:(dt + 1) * 512],
                    start=(p == 0), stop=(p == NP - 1))
            row = s * 512 + lqs * 128
            ost = ostp.tile([128, 512], f32, tag="ost", name=f"ost{s}_{lqs}_{dt}")
            nc.vector.tensor_copy(ost[:], u[:])
            nc.sync.dma_start(
                out=out_e[row:row + 128, dt * 512:(dt + 1) * 512], in_=ost[:])

        # ---- deadline-queue weave schedule ----
        # (deadline_iter, min_iter, unit); popped when deadline <= it+3, or
        # one filler per iteration once min_iter is reached.
        wq = []
        for lqt in range(1, 4):
            wq.append((4 * lqt - 1, 0, lambda lqt=lqt: k_unit(0, lqt)))
        for p in range(1, NP):
            for lqt in range(4):
                wq.append((16 * p + 4 * lqt - 1, 0, lambda p=p, lqt=lqt: k_unit(p, lqt)))
        for p in range(NP):
            for s in range(NS):
                if p == 0 and s == 0:
                    continue  # in lead
                wq.append((16 * (4 * s + p) - 1, 0, lambda p=p, s=s: q_unit(p, s)))
        for c in range(1, LC):
            wq.append((c, 0, lambda c=c: v_unit(c, 0)))
        for c in range(LC):
            wq.append((32 + c, 12, lambda c=c: v_unit(c, 1)))
        for s in range(NS - 1):
            for j, (lqs, dt) in enumerate((a, b) for a in range(4) for b in range(2)):
                wq.append((16 * (4 * s + 4) + 6 + 2 * j, 16 * (4 * s + 4) + 4,
                           lambda s=s, lqs=lqs, dt=dt: opj_unit(s, lqs, dt)))
        wq.sort(key=lambda t: t[0])

        # ---- normalize + pair-tile assembly per round ----
        def finalize(s, p, direct=False):
            # stage copies first: they feed the long denominator DMA chain
            stage = nrm.tile([65, 1024], f32, tag="stage", name=f"stage{s}_{p}")
            for h2 in range(2):
                nc.vector.tensor_copy(
                    stage[64:65, h2 * 512:(h2 + 1) * 512], pcs[h2][64:65, :])
            if direct:
                # last round: no next round needs pcs, multiply straight from
                # PSUM and skip the evacuation hop
                un = [pcs[0][0:64, :], pcs[1][0:64, :]]
            else:
                un = []
                for h2 in range(2):
                    ut = unp.tile([64, 512], bf16, tag="un", name=f"un{s}_{p}_{h2}")
                    nc.vector.tensor_copy(ut[:], pcs[h2][0:64, :])
                    un.append(ut[:])
            ct = ctpp.tile([128, 512], bf16, tag="ctp", name=f"ctp{s}_{p}")
            pbs = []
            if direct:
                # last round: latency-critical. Broadcast each denominator row
                # across partitions with a K=1 PE outer product (the PE is
                # idle and stays warm), then reciprocal on the spread rows —
                # no DMA round trips.
                for h2 in range(2):
                    nc.tensor.matmul(
                        util[h2][0:64, :], lhsT=ones65[64:65, :],
                        rhs=stage[64:65, h2 * 512:(h2 + 1) * 512],
                        start=True, stop=True)
                    pb = pbp.tile([64, 512], f32, tag="pbs", name=f"pbsd{h2}")
                    nc.vector.reciprocal_approx_fast(pb[:], util[h2][0:64, :])
                    pbs.append(pb)
            else:
                den = nrm.tile([2, 512], f32, tag="den", name=f"den{s}_{p}")
                nc.sync.dma_start(out=den[:], in_=stage[64:65, :])
                rec = nrm.tile([2, 512], f32, tag="rec", name=f"rec{s}_{p}")
                nc.vector.reciprocal_approx_fast(rec[:], den[:])
                # partition broadcast via DRAM bounce (0-stride partition
                # reads are only legal on DRAM APs)
                dr = drp.tile([2, 512], f32, tag="dr", name=f"dr{s}_{p}")
                nc.sync.dma_start(out=dr[:], in_=rec[:])
                for h2 in range(2):
                    pb = pbp.tile([64, 512], f32, tag="pbs", name=f"pbs{s}_{p}_{h2}")
                    rrow = dr[h2:h2 + 1, :]
                    rbc = bass.AP(
                        tensor=rrow.tensor, offset=rrow.offset,
                        ap=[[0, 64], [1, 512]])
                    nc.sync.dma_start(out=pb[:], in_=rbc)
                    pbs.append(pb)
            # odd head first: its SBUF->SBUF placement DMA overlaps h0's mult
            cto = ctop.tile([64, 512], bf16, tag="cto", name=f"cto{s}_{p}")
            nc.vector.tensor_mul(cto[:], un[1], pbs[1][:])
            nc.sync.dma_start(out=ct[64:128, :], in_=cto[:])
            nc.vector.tensor_mul(ct[0:64, :], un[0], pbs[0][:])
            ctp[(s, p)] = ct

        # ---- main pipeline ----
        sched = [(s, p, c) for s in range(NS) for p in range(NP) for c in range(LC)]
        NIT = len(sched)

        def emit_scores(it):
            s, p, c = sched[it]
            st = sts[it % 2]
            for h2 in range(2):
                base = 64 * h2
                nc.tensor.matmul(
                    st[:, h2 * 512:(h2 + 1) * 512],
                    lhsT=kT[p][base:base + 64, c * 128:(c + 1) * 128],
                    rhs=qT[p][base:base + 64, s * 512:(s + 1) * 512],
                    start=True, stop=True)

        # lead: minimal critical path to the first exp
        k_unit(0, 0)
        q_unit(0, 0)
        emit_scores(0)
        v_unit(0, 0)

        for it, (s, p, c) in enumerate(sched):
            if it + 1 < NIT:
                emit_scores(it + 1)
            et = etp.tile([128, 1024], bf16, tag="et", name=f"et{it}")
            nc.scalar.activation(et[:], sts[it % 2][:], ExpF)
            # weave before ctx: the PE chews projection/outproj work while
            # this iteration's exp finishes, instead of stalling at ctx
            emitted = 0
            while wq and wq[0][0] <= it + 3 and emitted < 2:
                wq.pop(0)[2]()
                emitted += 1
            if not emitted and wq and wq[0][1] <= it:
                wq.pop(0)[2]()
            for h2 in range(2):
                nc.tensor.matmul(
                    pcs[h2][:],
                    lhsT=vnat[:, c, (2 * p + h2) * VW:(2 * p + h2) * VW + VW],
                    rhs=et[:, h2 * 512:(h2 + 1) * 512],
                    start=(c == 0), stop=(c == LC - 1))
            if c == LC - 1:
                finalize(s, p, direct=(it == NIT - 1))

        while wq:
            wq.pop(0)[2]()

        # ---- tail: last strip's output projection ----
        for lqs in range(4):
            for dt in range(2):
                opj_unit(NS - 1, lqs, dt)

    nc.compile()
    return nc


_NC = None


def _get_nc():
    global _NC
    if _NC is None:
        _NC = build_graph()
    return _NC


# test harness can override, e.g. {"trace": True}
RUN_KWARGS: dict = {}
LAST_RESULTS = None


def make_in_maps(xq, xkv, Q, K, V, O):
    xq = np.asarray(xq, np.float32)
    xkv = np.asarray(xkv, np.float32)
    Q = np.asarray(Q, np.float32)
    K = np.asarray(K, np.float32)
    V = np.asarray(V, np.float32)
    O = np.asarray(O, np.float32)
    # cores 2b and 2b+1 share batch b's transposed activations; compute once
    xqT_c = [np.ascontiguousarray(xq[b].T).astype(np.float16) for b in range(B)]
    xkvT_c = [np.ascontiguousarray(xkv[b].T).astype(np.float16) for b in range(B)]
    in_maps = []
    for core in range(8):
        b, hg = divmod(core, 2)
        hs = slice(hg * HC, (hg + 1) * HC)
        in_maps.append({
            "xqT": xqT_c[b],
            "xkvT": xkvT_c[b],
            "Qw": np.ascontiguousarray(Q[:, hs, :].reshape(D, HA)).astype(np.float16),
            "Kw": np.ascontiguousarray(K[:, hs, :].reshape(D, HA)).astype(np.float16),
            "Vw": np.ascontiguousarray(V[:, hs, :].reshape(D, HA)).astype(np.float16),
            "OwT": np.ascontiguousarray(
                O[:, hs, :].reshape(D, HA).T).astype(ml_dtypes.bfloat16),
        })
    return in_maps


def kernel(xq, xkv, Q, K, V, O):
    global LAST_RESULTS
    nc = _get_nc()
    in_maps = make_in_maps(xq, xkv, Q, K, V, O)
    res = run_bass_kernel_spmd(nc, in_maps, core_ids=list(range(8)), **RUN_KWARGS)
    LAST_RESULTS = res
    outs = [np.asarray(res.results[c]["out"], np.float32) for c in range(8)]
    return np.stack([outs[2 * b] + outs[2 * b + 1] for b in range(B)], axis=0)


# revision 25
# speedup vs baseline: 1.2757x; 1.0593x over previous
"""Multi-head attention kernel for 8 TRN2 NeuronCores.

Reference: out = einsum('dha,blha->bld', O, softmax(q k^T) v) with
q/k/v = einsum('dha,bld->blha', W, x);  B=4, L=2048, D=1024, H=16, A=64.

Sharding: core c handles batch b = c//2 and head-group hg = c%2 (8 heads,
data parallel on B x tensor parallel on heads). Each core computes a partial
output [L, D] summed over its 8 heads; the host adds the two head-group
partials per batch.

Design (v3, ACT/PE co-scheduled):
  256 iterations over (strip s in 4 x pair p in 4 x lk-chunk c in 16) with
  512-wide lq strips. Per iteration: 2 score matmuls (K=64, one per head)
  fill one [128,1024] PSUM tile; ONE 1024-wide exp on ACT covers both
  heads; 2 ctx matmuls (K=128, M=65, ones row = free softmax denominators)
  accumulate into per-head [65,512] PSUM tiles. Scores run one iteration
  ahead so ACT (the ~285us exp stream) is the inner-loop pacer.

  x stays resident in SBUF (fp16, one [128,DC,L] tile per tensor, DMAed in
  512-column chunks so the first strip's k/q land ~15us in). Projections
  are "weave units" (k/q: [128,512] W^T@x; v: natural-layout x^T@Vw per
  lk-chunk per pair-half with built-in ones columns) dispatched by a
  deadline queue into per-iteration PE slack. Output projection for strip
  s runs as 4-matmul PSUM bursts woven after strip s completes.

  PSUM: 2x[128,1024] scores + 2x[65,512] ctx + 2x[128,512] utility = 8 banks.

  Normalize per (strip,pair): ctx evac to SBUF bf16, denominator rows
  joined on partition 64, reciprocal_approx_fast in place, then a direct
  partition-broadcast SBUF->SBUF DMA (0-stride source) issued from the
  idle GpSimd queue; odd head enters the pair tile via SBUF->SBUF DMA.

Measured on TRN2 (neuron-profile): see test.py. rel err ~8e-3.
"""

import sys

sys.path.insert(0, "/opt/trn_rl_repo")

from contextlib import ExitStack

import numpy as np
import ml_dtypes

import concourse.bass as bass  # noqa: F401
import concourse.tile as tile
from concourse import bacc, mybir
from concourse.bass_utils import run_bass_kernel_spmd

B, L, D, H, A = 4, 2048, 1024, 16, 64
HC = 8          # heads per core
NP = HC // 2    # head pairs per core
DC = D // 128   # d chunks
LC = L // 128   # lk chunks
HA = HC * A     # 512
SW = 512        # lq strip width
NS = L // SW    # 4 strips
VW = 65         # v block width per head in vnat (64 v + 1 ones)

f32 = mybir.dt.float32
bf16 = mybir.dt.bfloat16
f16 = mybir.dt.float16
ExpF = mybir.ActivationFunctionType.Exp


def build_graph():
    nc = bacc.Bacc("TRN2", target_bir_lowering=False, debug=False, num_devices=8)
    xqT_e = nc.dram_tensor("xqT", [D, L], f16, kind="ExternalInput").ap()
    xkvT_e = nc.dram_tensor("xkvT", [D, L], f16, kind="ExternalInput").ap()
    Qw_e = nc.dram_tensor("Qw", [D, HA], f16, kind="ExternalInput").ap()
    Kw_e = nc.dram_tensor("Kw", [D, HA], f16, kind="ExternalInput").ap()
    Vw_e = nc.dram_tensor("Vw", [D, HA], f16, kind="ExternalInput").ap()
    OwT_e = nc.dram_tensor("OwT", [HA, D], bf16, kind="ExternalInput").ap()
    out_e = nc.dram_tensor("out", [L, D], f32, kind="ExternalOutput").ap()

    with tile.TileContext(nc) as tc, ExitStack() as ctx:
        pers = ctx.enter_context(tc.tile_pool(name="pers", bufs=1))
        psp = ctx.enter_context(tc.tile_pool(name="psp", bufs=1, space="PSUM"))
        etp = ctx.enter_context(tc.tile_pool(name="etp", bufs=3))
        ctpp = ctx.enter_context(tc.tile_pool(name="ctpp", bufs=12))
        nrm = ctx.enter_context(tc.tile_pool(name="nrm", bufs=1))
        unp = ctx.enter_context(tc.tile_pool(name="unp", bufs=2))
        pbp = ctx.enter_context(tc.tile_pool(name="pbp", bufs=2))
        ctop = ctx.enter_context(tc.tile_pool(name="ctop", bufs=2))
        ostp = ctx.enter_context(tc.tile_pool(name="ostp", bufs=3))
        drp = ctx.enter_context(tc.tile_pool(name="drp", bufs=2, space="DRAM"))

        # ---- persistent SBUF ----
        xkv_t = pers.tile([128, DC, L], f16, tag="xkv", name="xkv")
        xq_t = pers.tile([128, DC, L], f16, tag="xq", name="xq")
        kT = [pers.tile([128, L], bf16, tag=f"kT{p}", name=f"kT{p}") for p in range(NP)]
        qT = [pers.tile([128, L], bf16, tag=f"qT{p}", name=f"qT{p}") for p in range(NP)]
        # vnat: [lk-part, lk-chunk, head-blocks of (64 v | 1 ones)]
        vnat = pers.tile([128, LC, HC * VW], bf16, tag="vnat", name="vnat")
        Kw_t = pers.tile([128, DC, HA], f16, tag="Kw", name="Kw")
        Qw_t = pers.tile([128, DC, HA], f16, tag="Qw", name="Qw")
        Vw_t = pers.tile([128, DC, HA], f16, tag="Vw", name="Vw")
        ow = [pers.tile([128, D], bf16, tag=f"ow{c}", name=f"ow{c}") for c in range(NP)]
        warm = pers.tile([1, 16], f32, tag="warm", name="warm")
        # ones row on partition 64: stationary for the K=1 broadcast matmul
        ones65 = pers.tile([65, 64], f32, tag="ones65", name="ones65")
        nc.vector.memset(ones65[:], 1.0)

        # ---- PSUM (exactly 8 banks) ----
        sts = [psp.tile([128, 1024], f32, tag=f"sts{i}", name=f"sts{i}") for i in range(2)]
        pcs = [psp.tile([65, 512], f32, tag=f"pcs{h}", name=f"pcs{h}") for h in range(2)]
        util = [psp.tile([128, 512], f32, tag=f"util{i}", name=f"util{i}") for i in range(2)]

        # ones columns of vnat: element h*65+64 of each (c, h) block
        v0 = vnat[:]
        ones_ap = bass.AP(
            tensor=v0.tensor,
            offset=v0.offset + 64,
            ap=[list(v0.ap[0]), [HC * VW, LC], [VW, HC]],
        )
        nc.vector.memset(ones_ap, 1.0)
        # warm the exp table during idle lead-in
        nc.vector.memset(warm[:], 0.0)
        nc.scalar.activation(warm[:], warm[:], ExpF)

        # ---- DMAs: column-chunked x, ordered for earliest first scores ----
        def w_ap(w_e):
            return bass.AP(
                tensor=w_e.tensor,
                offset=w_e.offset,
                ap=[[HA, 128], [128 * HA, DC], [1, HA]],
            )

        def x_cc_ap(x_e, cc):
            # [128 part, DC d-chunks, 512 cols] of x^T column-chunk cc
            return bass.AP(
                tensor=x_e.tensor,
                offset=x_e.offset + cc * 512,
                ap=[[L, 128], [128 * L, DC], [1, 512]],
            )

        nc.sync.dma_start(out=Kw_t[:], in_=w_ap(Kw_e))
        nc.sync.dma_start(out=xkv_t[:, :, 0:512], in_=x_cc_ap(xkvT_e, 0))
        nc.sync.dma_start(out=Qw_t[:], in_=w_ap(Qw_e))
        nc.sync.dma_start(out=xq_t[:, :, 0:512], in_=x_cc_ap(xqT_e, 0))
        nc.sync.dma_start(out=Vw_t[:], in_=w_ap(Vw_e))
        for cc in range(1, 4):
            nc.sync.dma_start(
                out=xkv_t[:, :, cc * 512:(cc + 1) * 512], in_=x_cc_ap(xkvT_e, cc))
        for cc in range(1, 4):
            nc.sync.dma_start(
                out=xq_t[:, :, cc * 512:(cc + 1) * 512], in_=x_cc_ap(xqT_e, cc))
        for c in range(NP):
            nc.sync.dma_start(out=ow[c][:], in_=OwT_e[c * 128:(c + 1) * 128, :])

        # ---- weave units ----
        util_i = [0]

        def next_util():
            u = util[util_i[0]]
            util_i[0] ^= 1
            return u

        def k_unit(p, lqt):
            u = next_util()
            for d in range(DC):
                nc.tensor.matmul(
                    u[:], lhsT=Kw_t[:, d, p * 128:(p + 1) * 128],
                    rhs=xkv_t[:, d, lqt * 512:(lqt + 1) * 512],
                    start=(d == 0), stop=(d == DC - 1))
            nc.vector.tensor_copy(kT[p][:, lqt * 512:(lqt + 1) * 512], u[:])

        def q_unit(p, s):
            u = next_util()
            for d in range(DC):
                nc.tensor.matmul(
                    u[:], lhsT=Qw_t[:, d, p * 128:(p + 1) * 128],
                    rhs=xq_t[:, d, s * 512:(s + 1) * 512],
                    start=(d == 0), stop=(d == DC - 1))
            nc.vector.tensor_copy(qT[p][:, s * 512:(s + 1) * 512], u[:])

        def v_unit(c, half):
            # natural-layout v for lk-chunk c, heads 4*half..4*half+3
            u = next_util()
            for d in range(DC):
                nc.tensor.matmul(
                    u[:, 0:256], lhsT=xkv_t[:, d, c * 128:(c + 1) * 128],
                    rhs=Vw_t[:, d, half * 256:(half + 1) * 256],
                    start=(d == 0), stop=(d == DC - 1))
            vc = vnat[:, c, :]
            dst = bass.AP(
                tensor=vc.tensor,
                offset=vc.offset + 4 * half * VW,
                ap=[list(vc.ap[0]), [VW, 4], [1, 64]],
            )
            ua = u[:]
            src = bass.AP(
                tensor=ua.tensor,
                offset=ua.offset,
                ap=[list(ua.ap[0]), [64, 4], [1, 64]],
            )
            nc.vector.tensor_copy(dst, src)

        ctp = {}

        def opj_unit(s, lqs, dt):
            u = next_util()
            for p in range(NP):
                nc.tensor.matmul(
                    u[:], lhsT=ctp[(s, p)][:, lqs * 128:(lqs + 1) * 128],
                    rhs=ow[p][:, dt * 512:(dt + 1) * 512],
                    start=(p == 0), stop=(p == NP - 1))
            row = s * 512 + lqs * 128
            ost = ostp.tile([128, 512], f32, tag="ost", name=f"ost{s}_{lqs}_{dt}")
            nc.vector.tensor_copy(ost[:], u[:])
            nc.sync.dma_start(
                out=out_e[row:row + 128, dt * 512:(dt + 1) * 512], in_=ost[:])

        # ---- deadline-queue weave schedule ----
        # (deadline_iter, min_iter, unit); popped when deadline <= it+3, or
        # one filler per iteration once min_iter is reached.
        wq = []
        for lqt in range(1, 4):
            wq.append((4 * lqt - 1, 0, lambda lqt=lqt: k_unit(0, lqt)))
        for p in range(1, NP):
            for lqt in range(4):
                wq.append((16 * p + 4 * lqt - 1, 0, lambda p=p, lqt=lqt: k_unit(p, lqt)))
        for p in range(NP):
            for s in range(NS):
                if p == 0 and s == 0:
                    continue  # in lead
                wq.append((16 * (4 * s + p) - 1, 0, lambda p=p, s=s: q_unit(p, s)))
        for c in range(1, LC):
            wq.append((c, 0, lambda c=c: v_unit(c, 0)))
        for c in range(LC):
            wq.append((32 + c, 12, lambda c=c: v_unit(c, 1)))
        for s in range(NS - 1):
            # spread across the whole following strip window so the PE keeps
            # a filler unit in most iterations
            for j, (lqs, dt) in enumerate((a, b) for a in range(4) for b in range(2)):
                dl = 16 * (4 * s + 4) + 6 + 7 * j
                wq.append((dl, dl - 2,
                           lambda s=s, lqs=lqs, dt=dt: opj_unit(s, lqs, dt)))
        wq.sort(key=lambda t: t[0])

        # ---- normalize + pair-tile assembly per round ----
        def finalize(s, p, direct=False):
            # stage copies first: they feed the long denominator DMA chain
            stage = nrm.tile([65, 1024], f32, tag="stage", name=f"stage{s}_{p}")
            for h2 in range(2):
                nc.vector.tensor_copy(
                    stage[64:65, h2 * 512:(h2 + 1) * 512], pcs[h2][64:65, :])
            if direct:
                # last round: no next round needs pcs, multiply straight from
                # PSUM and skip the evacuation hop
                un = [pcs[0][0:64, :], pcs[1][0:64, :]]
            else:
                un = []
                for h2 in range(2):
                    ut = unp.tile([64, 512], bf16, tag="un", name=f"un{s}_{p}_{h2}")
                    nc.vector.tensor_copy(ut[:], pcs[h2][0:64, :])
                    un.append(ut[:])
            ct = ctpp.tile([128, 512], bf16, tag="ctp", name=f"ctp{s}_{p}")
            pbs = []
            if direct:
                # last round: latency-critical. Broadcast each denominator row
                # across partitions with a K=1 PE outer product (the PE is
                # idle and stays warm), then reciprocal on the spread rows —
                # no DMA round trips.
                for h2 in range(2):
                    nc.tensor.matmul(
                        util[h2][0:64, :], lhsT=ones65[64:65, :],
                        rhs=stage[64:65, h2 * 512:(h2 + 1) * 512],
                        start=True, stop=True)
                    pb = pbp.tile([64, 512], f32, tag="pbs", name=f"pbsd{h2}")
                    nc.vector.reciprocal_approx_fast(pb[:], util[h2][0:64, :])
                    pbs.append(pb)
            else:
                den = nrm.tile([2, 512], f32, tag="den", name=f"den{s}_{p}")
                nc.sync.dma_start(out=den[:], in_=stage[64:65, :])
                rec = nrm.tile([2, 512], f32, tag="rec", name=f"rec{s}_{p}")
                nc.vector.reciprocal_approx_fast(rec[:], den[:])
                # partition broadcast via DRAM bounce (0-stride partition
                # reads are only legal on DRAM APs)
                dr = drp.tile([2, 512], f32, tag="dr", name=f"dr{s}_{p}")
                nc.sync.dma_start(out=dr[:], in_=rec[:])
                for h2 in range(2):
                    pb = pbp.tile([64, 512], f32, tag="pbs", name=f"pbs{s}_{p}_{h2}")
                    rrow = dr[h2:h2 + 1, :]
                    rbc = bass.AP(
                        tensor=rrow.tensor, offset=rrow.offset,
                        ap=[[0, 64], [1, 512]])
                    nc.sync.dma_start(out=pb[:], in_=rbc)
                    pbs.append(pb)
            # odd head first: its SBUF->SBUF placement DMA overlaps h0's mult
            cto = ctop.tile([64, 512], bf16, tag="cto", name=f"cto{s}_{p}")
            nc.vector.tensor_mul(cto[:], un[1], pbs[1][:])
            nc.sync.dma_start(out=ct[64:128, :], in_=cto[:])
            nc.vector.tensor_mul(ct[0:64, :], un[0], pbs[0][:])
            ctp[(s, p)] = ct

        # ---- main pipeline ----
        sched = [(s, p, c) for s in range(NS) for p in range(NP) for c in range(LC)]
        NIT = len(sched)

        def emit_scores(it):
            s, p, c = sched[it]
            st = sts[it % 2]
            for h2 in range(2):
                base = 64 * h2
                nc.tensor.matmul(
                    st[:, h2 * 512:(h2 + 1) * 512],
                    lhsT=kT[p][base:base + 64, c * 128:(c + 1) * 128],
                    rhs=qT[p][base:base + 64, s * 512:(s + 1) * 512],
                    start=True, stop=True)

        # lead: minimal critical path to the first exp
        k_unit(0, 0)
        q_unit(0, 0)
        emit_scores(0)
        v_unit(0, 0)

        for it, (s, p, c) in enumerate(sched):
            if it + 1 < NIT:
                emit_scores(it + 1)
            et = etp.tile([128, 1024], bf16, tag="et", name=f"et{it}")
            nc.scalar.activation(et[:], sts[it % 2][:], ExpF)
            # weave before ctx: the PE chews projection/outproj work while
            # this iteration's exp finishes, instead of stalling at ctx
            emitted = 0
            while wq and wq[0][0] <= it + 3 and emitted < 2:
                wq.pop(0)[2]()
                emitted += 1
            if not emitted and wq and wq[0][1] <= it:
                wq.pop(0)[2]()
            for h2 in range(2):
                nc.tensor.matmul(
                    pcs[h2][:],
                    lhsT=vnat[:, c, (2 * p + h2) * VW:(2 * p + h2) * VW + VW],
                    rhs=et[:, h2 * 512:(h2 + 1) * 512],
                    start=(c == 0), stop=(c == LC - 1))
            if c == LC - 1:
                finalize(s, p, direct=(it == NIT - 1))

        while wq:
            wq.pop(0)[2]()

        # ---- tail: last strip's output projection ----
        for lqs in range(4):
            for dt in range(2):
                opj_unit(NS - 1, lqs, dt)

    nc.compile()
    return nc


_NC = None


def _get_nc():
    global _NC
    if _NC is None:
        _NC = build_graph()
    return _NC


# test harness can override, e.g. {"trace": True}
RUN_KWARGS: dict = {}
LAST_RESULTS = None


def make_in_maps(xq, xkv, Q, K, V, O):
    xq = np.asarray(xq, np.float32)
    xkv = np.asarray(xkv, np.float32)
    Q = np.asarray(Q, np.float32)
    K = np.asarray(K, np.float32)
    V = np.asarray(V, np.float32)
    O = np.asarray(O, np.float32)
    # cores 2b and 2b+1 share batch b's transposed activations; compute once
    xqT_c = [np.ascontiguousarray(xq[b].T).astype(np.float16) for b in range(B)]
    xkvT_c = [np.ascontiguousarray(xkv[b].T).astype(np.float16) for b in range(B)]
    in_maps = []
    for core in range(8):
        b, hg = divmod(core, 2)
        hs = slice(hg * HC, (hg + 1) * HC)
        in_maps.append({
            "xqT": xqT_c[b],
            "xkvT": xkvT_c[b],
            "Qw": np.ascontiguousarray(Q[:, hs, :].reshape(D, HA)).astype(np.float16),
            "Kw": np.ascontiguousarray(K[:, hs, :].reshape(D, HA)).astype(np.float16),
            "Vw": np.ascontiguousarray(V[:, hs, :].reshape(D, HA)).astype(np.float16),
            "OwT": np.ascontiguousarray(
                O[:, hs, :].reshape(D, HA).T).astype(ml_dtypes.bfloat16),
        })
    return in_maps


def kernel(xq, xkv, Q, K, V, O):
    global LAST_RESULTS
    nc = _get_nc()
    in_maps = make_in_maps(xq, xkv, Q, K, V, O)
    res = run_bass_kernel_spmd(nc, in_maps, core_ids=list(range(8)), **RUN_KWARGS)
    LAST_RESULTS = res
    outs = [np.asarray(res.results[c]["out"], np.float32) for c in range(8)]
    return np.stack([outs[2 * b] + outs[2 * b + 1] for b in range(B)], axis=0)


# revision 30
# speedup vs baseline: 1.3029x; 1.0214x over previous
"""Multi-head attention kernel for 8 TRN2 NeuronCores.

Reference: out = einsum('dha,blha->bld', O, softmax(q k^T) v) with
q/k/v = einsum('dha,bld->blha', W, x);  B=4, L=2048, D=1024, H=16, A=64.

Sharding: core c handles batch b = c//2 and head-group hg = c%2 (8 heads,
data parallel on B x tensor parallel on heads). Each core computes a partial
output [L, D] summed over its 8 heads; the host adds the two head-group
partials per batch.

Design (final, ACT/PE co-scheduled):
  256 iterations over (strip s in 4 x pair p in 4 x lk-chunk c in 16) with
  512-wide lq strips. Per iteration: 2 score matmuls (K=64, one per head)
  fill one [128,1024] PSUM tile; ONE 1024-wide exp on ACT covers both
  heads; 2 ctx matmuls (K=128, M=65, ones row = free softmax denominators)
  accumulate into per-head [65,512] PSUM tiles. Scores run one iteration
  ahead so ACT (the ~285us exp stream) is the inner-loop pacer.

  x stays resident in SBUF (fp16, one [128,DC,L] tile per tensor, DMAed in
  512-column chunks so the first strip's k/q land ~15us in). Projections
  are "weave units" (k/q: [128,512] W^T@x; v: natural-layout x^T@Vw per
  lk-chunk per pair-half with built-in ones columns) dispatched by a
  deadline queue into per-iteration PE slack. Output projection for strip
  s runs as 4-matmul PSUM bursts woven after strip s completes.

  PSUM: 2x[128,1024] scores + 2x[65,512] ctx + 2x[128,512] utility = 8 banks.

  Normalize per (strip,pair): denominator rows joined on partition 64,
  reciprocal_approx_fast on a [2,512] gather, DRAM-bounce partition
  broadcast, DVE multiplies; the odd head enters the pair tile via
  SBUF->SBUF DMA. The LAST round instead broadcasts each denominator row
  with a K=1 PE outer product and multiplies straight from PSUM — no DMA
  round trips on the tail-critical chain, and the PE stays warm for the
  final output-projection bursts.

Measured on TRN2 (neuron-profile): 396-430us cool thermal state
(vs 487us baseline; device clocks vary ~20% with thermal state),
rel err 8.0e-3.
"""

import sys

sys.path.insert(0, "/opt/trn_rl_repo")

from contextlib import ExitStack

import numpy as np
import ml_dtypes

import concourse.bass as bass  # noqa: F401
import concourse.tile as tile
from concourse import bacc, mybir
from concourse.bass_utils import run_bass_kernel_spmd

B, L, D, H, A = 4, 2048, 1024, 16, 64
HC = 8          # heads per core
NP = HC // 2    # head pairs per core
DC = D // 128   # d chunks
LC = L // 128   # lk chunks
HA = HC * A     # 512
SW = 512        # lq strip width
NS = L // SW    # 4 strips
VW = 65         # v block width per head in vnat (64 v + 1 ones)

f32 = mybir.dt.float32
bf16 = mybir.dt.bfloat16
f16 = mybir.dt.float16
ExpF = mybir.ActivationFunctionType.Exp


def build_graph():
    nc = bacc.Bacc("TRN2", target_bir_lowering=False, debug=False, num_devices=8)
    xqT_e = nc.dram_tensor("xqT", [D, L], f16, kind="ExternalInput").ap()
    xkvT_e = nc.dram_tensor("xkvT", [D, L], f16, kind="ExternalInput").ap()
    Qw_e = nc.dram_tensor("Qw", [D, HA], f16, kind="ExternalInput").ap()
    Kw_e = nc.dram_tensor("Kw", [D, HA], f16, kind="ExternalInput").ap()
    Vw_e = nc.dram_tensor("Vw", [D, HA], f16, kind="ExternalInput").ap()
    OwT_e = nc.dram_tensor("OwT", [HA, D], bf16, kind="ExternalInput").ap()
    out_e = nc.dram_tensor("out", [L, D], f32, kind="ExternalOutput").ap()

    with tile.TileContext(nc) as tc, ExitStack() as ctx:
        pers = ctx.enter_context(tc.tile_pool(name="pers", bufs=1))
        psp = ctx.enter_context(tc.tile_pool(name="psp", bufs=1, space="PSUM"))
        etp = ctx.enter_context(tc.tile_pool(name="etp", bufs=3))
        ctpp = ctx.enter_context(tc.tile_pool(name="ctpp", bufs=12))
        nrm = ctx.enter_context(tc.tile_pool(name="nrm", bufs=1))
        unp = ctx.enter_context(tc.tile_pool(name="unp", bufs=2))
        pbp = ctx.enter_context(tc.tile_pool(name="pbp", bufs=2))
        ctop = ctx.enter_context(tc.tile_pool(name="ctop", bufs=2))
        ostp = ctx.enter_context(tc.tile_pool(name="ostp", bufs=3))
        drp = ctx.enter_context(tc.tile_pool(name="drp", bufs=2, space="DRAM"))

        # ---- persistent SBUF ----
        xkv_t = pers.tile([128, DC, L], f16, tag="xkv", name="xkv")
        xq_t = pers.tile([128, DC, L], f16, tag="xq", name="xq")
        kT = [pers.tile([128, L], bf16, tag=f"kT{p}", name=f"kT{p}") for p in range(NP)]
        qT = [pers.tile([128, L], bf16, tag=f"qT{p}", name=f"qT{p}") for p in range(NP)]
        # vnat: [lk-part, lk-chunk, head-blocks of (64 v | 1 ones)]
        vnat = pers.tile([128, LC, HC * VW], bf16, tag="vnat", name="vnat")
        Kw_t = pers.tile([128, DC, HA], f16, tag="Kw", name="Kw")
        Qw_t = pers.tile([128, DC, HA], f16, tag="Qw", name="Qw")
        Vw_t = pers.tile([128, DC, HA], f16, tag="Vw", name="Vw")
        ow = [pers.tile([128, D], bf16, tag=f"ow{c}", name=f"ow{c}") for c in range(NP)]
        warm = pers.tile([1, 16], f32, tag="warm", name="warm")
        # ones row on partition 64: stationary for the K=1 broadcast matmul
        ones65 = pers.tile([65, 64], f32, tag="ones65", name="ones65")
        nc.vector.memset(ones65[:], 1.0)

        # ---- PSUM (exactly 8 banks) ----
        sts = [psp.tile([128, 1024], f32, tag=f"sts{i}", name=f"sts{i}") for i in range(2)]
        pcs = [psp.tile([65, 512], f32, tag=f"pcs{h}", name=f"pcs{h}") for h in range(2)]
        util = [psp.tile([128, 512], f32, tag=f"util{i}", name=f"util{i}") for i in range(2)]

        # ones columns of vnat: element h*65+64 of each (c, h) block
        v0 = vnat[:]
        ones_ap = bass.AP(
            tensor=v0.tensor,
            offset=v0.offset + 64,
            ap=[list(v0.ap[0]), [HC * VW, LC], [VW, HC]],
        )
        nc.vector.memset(ones_ap, 1.0)
        # warm the exp table during idle lead-in
        nc.vector.memset(warm[:], 0.0)
        nc.scalar.activation(warm[:], warm[:], ExpF)

        # ---- DMAs: column-chunked x, ordered for earliest first scores ----
        def w_ap(w_e):
            return bass.AP(
                tensor=w_e.tensor,
                offset=w_e.offset,
                ap=[[HA, 128], [128 * HA, DC], [1, HA]],
            )

        def x_cc_ap(x_e, cc):
            # [128 part, DC d-chunks, 512 cols] of x^T column-chunk cc
            return bass.AP(
                tensor=x_e.tensor,
                offset=x_e.offset + cc * 512,
                ap=[[L, 128], [128 * L, DC], [1, 512]],
            )

        def w_slice_ap(w_e, lo, n):
            return bass.AP(
                tensor=w_e.tensor,
                offset=w_e.offset + lo,
                ap=[[HA, 128], [128 * HA, DC], [1, n]],
            )

        # pair-0 weight slices first: the first k/q units only need 128 of
        # the 512 weight columns, so the pipeline ignites ~6us earlier
        nc.sync.dma_start(out=Kw_t[:, :, 0:128], in_=w_slice_ap(Kw_e, 0, 128))
        nc.sync.dma_start(out=xkv_t[:, :, 0:512], in_=x_cc_ap(xkvT_e, 0))
        nc.sync.dma_start(out=Vw_t[:], in_=w_ap(Vw_e))
        nc.sync.dma_start(out=Qw_t[:, :, 0:128], in_=w_slice_ap(Qw_e, 0, 128))
        nc.sync.dma_start(out=xq_t[:, :, 0:512], in_=x_cc_ap(xqT_e, 0))
        for cc in range(1, 4):
            nc.sync.dma_start(
                out=xkv_t[:, :, cc * 512:(cc + 1) * 512], in_=x_cc_ap(xkvT_e, cc))
        nc.sync.dma_start(out=Kw_t[:, :, 128:512], in_=w_slice_ap(Kw_e, 128, 384))
        nc.sync.dma_start(out=Qw_t[:, :, 128:512], in_=w_slice_ap(Qw_e, 128, 384))
        for cc in range(1, 4):
            nc.sync.dma_start(
                out=xq_t[:, :, cc * 512:(cc + 1) * 512], in_=x_cc_ap(xqT_e, cc))
        for c in range(NP):
            nc.sync.dma_start(out=ow[c][:], in_=OwT_e[c * 128:(c + 1) * 128, :])

        # ---- weave units ----
        util_i = [0]

        def next_util():
            u = util[util_i[0]]
            util_i[0] ^= 1
            return u

        def k_unit(p, lqt):
            u = next_util()
            for d in range(DC):
                nc.tensor.matmul(
                    u[:], lhsT=Kw_t[:, d, p * 128:(p + 1) * 128],
                    rhs=xkv_t[:, d, lqt * 512:(lqt + 1) * 512],
                    start=(d == 0), stop=(d == DC - 1))
            nc.vector.tensor_copy(kT[p][:, lqt * 512:(lqt + 1) * 512], u[:])

        def q_unit(p, s):
            u = next_util()
            for d in range(DC):
                nc.tensor.matmul(
                    u[:], lhsT=Qw_t[:, d, p * 128:(p + 1) * 128],
                    rhs=xq_t[:, d, s * 512:(s + 1) * 512],
                    start=(d == 0), stop=(d == DC - 1))
            nc.vector.tensor_copy(qT[p][:, s * 512:(s + 1) * 512], u[:])

        def v_unit(c, half):
            # natural-layout v for lk-chunk c, heads 4*half..4*half+3
            u = next_util()
            for d in range(DC):
                nc.tensor.matmul(
                    u[:, 0:256], lhsT=xkv_t[:, d, c * 128:(c + 1) * 128],
                    rhs=Vw_t[:, d, half * 256:(half + 1) * 256],
                    start=(d == 0), stop=(d == DC - 1))
            vc = vnat[:, c, :]
            dst = bass.AP(
                tensor=vc.tensor,
                offset=vc.offset + 4 * half * VW,
                ap=[list(vc.ap[0]), [VW, 4], [1, 64]],
            )
            ua = u[:]
            src = bass.AP(
                tensor=ua.tensor,
                offset=ua.offset,
                ap=[list(ua.ap[0]), [64, 4], [1, 64]],
            )
            nc.vector.tensor_copy(dst, src)

        ctp = {}

        def opj_unit(s, lqs, dt):
            u = next_util()
            for p in range(NP):
                nc.tensor.matmul(
                    u[:], lhsT=ctp[(s, p)][:, lqs * 128:(lqs + 1) * 128],
                    rhs=ow[p][:, dt * 512:(dt + 1) * 512],
                    start=(p == 0), stop=(p == NP - 1))
            row = s * 512 + lqs * 128
            ost = ostp.tile([128, 512], f32, tag="ost", name=f"ost{s}_{lqs}_{dt}")
            nc.vector.tensor_copy(ost[:], u[:])
            nc.sync.dma_start(
                out=out_e[row:row + 128, dt * 512:(dt + 1) * 512], in_=ost[:])

        # ---- deadline-queue weave schedule ----
        # (deadline_iter, min_iter, unit); popped when deadline <= it+3, or
        # one filler per iteration once min_iter is reached.
        wq = []
        for lqt in range(1, 4):
            wq.append((4 * lqt - 1, 0, lambda lqt=lqt: k_unit(0, lqt)))
        for p in range(1, NP):
            for lqt in range(4):
                wq.append((16 * p + 4 * lqt - 1, 0, lambda p=p, lqt=lqt: k_unit(p, lqt)))
        for p in range(NP):
            for s in range(NS):
                if p == 0 and s == 0:
                    continue  # in lead
                wq.append((16 * (4 * s + p) - 1, 0, lambda p=p, s=s: q_unit(p, s)))
        for c in range(2, LC):
            wq.append((c, 0, lambda c=c: v_unit(c, 0)))
        for c in range(LC):
            wq.append((32 + c, 12, lambda c=c: v_unit(c, 1)))
        for s in range(NS - 1):
            # spread across the whole following strip window so the PE keeps
            # a filler unit in most iterations
            for j, (lqs, dt) in enumerate((a, b) for a in range(4) for b in range(2)):
                dl = 16 * (4 * s + 4) + 6 + 7 * j
                wq.append((dl, dl - 2,
                           lambda s=s, lqs=lqs, dt=dt: opj_unit(s, lqs, dt)))
        wq.sort(key=lambda t: t[0])

        # ---- normalize + pair-tile assembly per round ----
        def finalize(s, p, direct=False):
            # stage copies first: they feed the long denominator DMA chain
            stage = nrm.tile([65, 1024], f32, tag="stage", name=f"stage{s}_{p}")
            for h2 in range(2):
                nc.vector.tensor_copy(
                    stage[64:65, h2 * 512:(h2 + 1) * 512], pcs[h2][64:65, :])
            if direct:
                # last round: no next round needs pcs, multiply straight from
                # PSUM and skip the evacuation hop
                un = [pcs[0][0:64, :], pcs[1][0:64, :]]
            else:
                un = []
                for h2 in range(2):
                    ut = unp.tile([64, 512], bf16, tag="un", name=f"un{s}_{p}_{h2}")
                    nc.vector.tensor_copy(ut[:], pcs[h2][0:64, :])
                    un.append(ut[:])
            ct = ctpp.tile([128, 512], bf16, tag="ctp", name=f"ctp{s}_{p}")
            pbs = []
            if direct:
                # last round: latency-critical. Broadcast each denominator row
                # across partitions with a K=1 PE outer product (the PE is
                # idle and stays warm), then reciprocal on the spread rows —
                # no DMA round trips.
                for h2 in range(2):
                    nc.tensor.matmul(
                        util[h2][0:64, :], lhsT=ones65[64:65, :],
                        rhs=stage[64:65, h2 * 512:(h2 + 1) * 512],
                        start=True, stop=True)
                    pb = pbp.tile([64, 512], f32, tag="pbs", name=f"pbsd{h2}")
                    nc.vector.reciprocal_approx_fast(pb[:], util[h2][0:64, :])
                    pbs.append(pb)
            else:
                den = nrm.tile([2, 512], f32, tag="den", name=f"den{s}_{p}")
                nc.sync.dma_start(out=den[:], in_=stage[64:65, :])
                rec = nrm.tile([2, 512], f32, tag="rec", name=f"rec{s}_{p}")
                nc.vector.reciprocal_approx_fast(rec[:], den[:])
                # partition broadcast via DRAM bounce (0-stride partition
                # reads are only legal on DRAM APs)
                dr = drp.tile([2, 512], f32, tag="dr", name=f"dr{s}_{p}")
                nc.sync.dma_start(out=dr[:], in_=rec[:])
                for h2 in range(2):
                    pb = pbp.tile([64, 512], f32, tag="pbs", name=f"pbs{s}_{p}_{h2}")
                    rrow = dr[h2:h2 + 1, :]
                    rbc = bass.AP(
                        tensor=rrow.tensor, offset=rrow.offset,
                        ap=[[0, 64], [1, 512]])
                    nc.sync.dma_start(out=pb[:], in_=rbc)
                    pbs.append(pb)
            # odd head first: its SBUF->SBUF placement DMA overlaps h0's mult
            cto = ctop.tile([64, 512], bf16, tag="cto", name=f"cto{s}_{p}")
            nc.vector.tensor_mul(cto[:], un[1], pbs[1][:])
            nc.sync.dma_start(out=ct[64:128, :], in_=cto[:])
            nc.vector.tensor_mul(ct[0:64, :], un[0], pbs[0][:])
            ctp[(s, p)] = ct

        # ---- main pipeline ----
        sched = [(s, p, c) for s in range(NS) for p in range(NP) for c in range(LC)]
        NIT = len(sched)

        def emit_scores(it):
            s, p, c = sched[it]
            st = sts[it % 2]
            for h2 in range(2):
                base = 64 * h2
                nc.tensor.matmul(
                    st[:, h2 * 512:(h2 + 1) * 512],
                    lhsT=kT[p][base:base + 64, c * 128:(c + 1) * 128],
                    rhs=qT[p][base:base + 64, s * 512:(s + 1) * 512],
                    start=True, stop=True)

        # lead: minimal critical path to the first exp; v halves slot into
        # the DMA-arrival gaps so the PE never idles waiting for xq
        k_unit(0, 0)
        v_unit(0, 0)
        v_unit(1, 0)
        q_unit(0, 0)
        emit_scores(0)

        for it, (s, p, c) in enumerate(sched):
            if it + 1 < NIT:
                emit_scores(it + 1)
            et = etp.tile([128, 1024], bf16, tag="et", name=f"et{it}")
            nc.scalar.activation(et[:], sts[it % 2][:], ExpF)
            # weave before ctx: the PE chews projection/outproj work while
            # this iteration's exp finishes, instead of stalling at ctx
            emitted = 0
            while wq and wq[0][0] <= it + 3 and emitted < 2:
                wq.pop(0)[2]()
                emitted += 1
            if not emitted and wq and wq[0][1] <= it:
                wq.pop(0)[2]()
            for h2 in range(2):
                nc.tensor.matmul(
                    pcs[h2][:],
                    lhsT=vnat[:, c, (2 * p + h2) * VW:(2 * p + h2) * VW + VW],
                    rhs=et[:, h2 * 512:(h2 + 1) * 512],
                    start=(c == 0), stop=(c == LC - 1))
            if c == LC - 1:
                finalize(s, p, direct=(it == NIT - 1))

        while wq:
            wq.pop(0)[2]()

        # ---- tail: last strip's output projection ----
        for lqs in range(4):
            for dt in range(2):
                opj_unit(NS - 1, lqs, dt)

    nc.compile()
    return nc


_NC = None


def _get_nc():
    global _NC
    if _NC is None:
        _NC = build_graph()
    return _NC


# test harness can override, e.g. {"trace": True}
RUN_KWARGS: dict = {}
LAST_RESULTS = None


def make_in_maps(xq, xkv, Q, K, V, O):
    xq = np.asarray(xq, np.float32)
    xkv = np.asarray(xkv, np.float32)
    Q = np.asarray(Q, np.float32)
    K = np.asarray(K, np.float32)
    V = np.asarray(V, np.float32)
    O = np.asarray(O, np.float32)
    # cores 2b and 2b+1 share batch b's transposed activations; compute once
    xqT_c = [np.ascontiguousarray(xq[b].T).astype(np.float16) for b in range(B)]
    xkvT_c = [np.ascontiguousarray(xkv[b].T).astype(np.float16) for b in range(B)]
    in_maps = []
    for core in range(8):
        b, hg = divmod(core, 2)
        hs = slice(hg * HC, (hg + 1) * HC)
        in_maps.append({
            "xqT": xqT_c[b],
            "xkvT": xkvT_c[b],
            "Qw": np.ascontiguousarray(Q[:, hs, :].reshape(D, HA)).astype(np.float16),
            "Kw": np.ascontiguousarray(K[:, hs, :].reshape(D, HA)).astype(np.float16),
            "Vw": np.ascontiguousarray(V[:, hs, :].reshape(D, HA)).astype(np.float16),
            "OwT": np.ascontiguousarray(
                O[:, hs, :].reshape(D, HA).T).astype(ml_dtypes.bfloat16),
        })
    return in_maps


def kernel(xq, xkv, Q, K, V, O):
    global LAST_RESULTS
    nc = _get_nc()
    in_maps = make_in_maps(xq, xkv, Q, K, V, O)
    res = run_bass_kernel_spmd(nc, in_maps, core_ids=list(range(8)), **RUN_KWARGS)
    LAST_RESULTS = res
    outs = [np.asarray(res.results[c]["out"], np.float32) for c in range(8)]
    return np.stack([outs[2 * b] + outs[2 * b + 1] for b in range(B)], axis=0)


# revision 32
# speedup vs baseline: 1.3035x; 1.0005x over previous
"""Multi-head attention kernel for 8 TRN2 NeuronCores.

Reference: out = einsum('dha,blha->bld', O, softmax(q k^T) v) with
q/k/v = einsum('dha,bld->blha', W, x);  B=4, L=2048, D=1024, H=16, A=64.

Sharding: core c handles batch b = c//2 and head-group hg = c%2 (8 heads,
data parallel on B x tensor parallel on heads). Each core computes a partial
output [L, D] summed over its 8 heads; the host adds the two head-group
partials per batch.

Design (final, ACT/PE co-scheduled):
  256 iterations over (strip s in 4 x pair p in 4 x lk-chunk c in 16) with
  512-wide lq strips. Per iteration: 2 score matmuls (K=64, one per head)
  fill one [128,1024] PSUM tile; ONE 1024-wide exp on ACT covers both
  heads; 2 ctx matmuls (K=128, M=65, ones row = free softmax denominators)
  accumulate into per-head [65,512] PSUM tiles. Scores run one iteration
  ahead so ACT (the ~285us exp stream) is the inner-loop pacer.

  x stays resident in SBUF (fp16, one [128,DC,L] tile per tensor, DMAed in
  512-column chunks so the first strip's k/q land ~15us in). Projections
  are "weave units" (k/q: [128,512] W^T@x; v: natural-layout x^T@Vw per
  lk-chunk per pair-half with built-in ones columns) dispatched by a
  deadline queue into per-iteration PE slack. Output projection for strip
  s runs as 4-matmul PSUM bursts woven after strip s completes.

  PSUM: 2x[128,1024] scores + 2x[65,512] ctx + 2x[128,512] utility = 8 banks.

  Normalize per (strip,pair): denominator rows joined on partition 64,
  reciprocal_approx_fast on a [2,512] gather, DRAM-bounce partition
  broadcast, DVE multiplies; the odd head enters the pair tile via
  SBUF->SBUF DMA. The LAST round instead broadcasts each denominator row
  with a K=1 PE outer product and multiplies straight from PSUM — no DMA
  round trips on the tail-critical chain, and the PE stays warm for the
  final output-projection bursts.

Measured on TRN2 (neuron-profile): 388us cool thermal state
(vs 487us baseline; device clocks vary ~20% with thermal state),
rel err 8.0e-3.
"""

import sys

sys.path.insert(0, "/opt/trn_rl_repo")

from contextlib import ExitStack

import numpy as np
import ml_dtypes

import concourse.bass as bass  # noqa: F401
import concourse.tile as tile
from concourse import bacc, mybir
from concourse.bass_utils import run_bass_kernel_spmd

B, L, D, H, A = 4, 2048, 1024, 16, 64
HC = 8          # heads per core
NP = HC // 2    # head pairs per core
DC = D // 128   # d chunks
LC = L // 128   # lk chunks
HA = HC * A     # 512
SW = 512        # lq strip width
NS = L // SW    # 4 strips
VW = 65         # v block width per head in vnat (64 v + 1 ones)

f32 = mybir.dt.float32
bf16 = mybir.dt.bfloat16
f16 = mybir.dt.float16
ExpF = mybir.ActivationFunctionType.Exp


def build_graph():
    nc = bacc.Bacc("TRN2", target_bir_lowering=False, debug=False, num_devices=8)
    xqT_e = nc.dram_tensor("xqT", [D, L], f16, kind="ExternalInput").ap()
    xkvT_e = nc.dram_tensor("xkvT", [D, L], f16, kind="ExternalInput").ap()
    Qw_e = nc.dram_tensor("Qw", [D, HA], f16, kind="ExternalInput").ap()
    Kw_e = nc.dram_tensor("Kw", [D, HA], f16, kind="ExternalInput").ap()
    Vw_e = nc.dram_tensor("Vw", [D, HA], f16, kind="ExternalInput").ap()
    OwT_e = nc.dram_tensor("OwT", [HA, D], bf16, kind="ExternalInput").ap()
    out_e = nc.dram_tensor("out", [L, D], f32, kind="ExternalOutput").ap()

    with tile.TileContext(nc) as tc, ExitStack() as ctx:
        pers = ctx.enter_context(tc.tile_pool(name="pers", bufs=1))
        psp = ctx.enter_context(tc.tile_pool(name="psp", bufs=1, space="PSUM"))
        etp = ctx.enter_context(tc.tile_pool(name="etp", bufs=3))
        ctpp = ctx.enter_context(tc.tile_pool(name="ctpp", bufs=12))
        nrm = ctx.enter_context(tc.tile_pool(name="nrm", bufs=1))
        unp = ctx.enter_context(tc.tile_pool(name="unp", bufs=2))
        pbp = ctx.enter_context(tc.tile_pool(name="pbp", bufs=2))
        ctop = ctx.enter_context(tc.tile_pool(name="ctop", bufs=2))
        ostp = ctx.enter_context(tc.tile_pool(name="ostp", bufs=5))
        drp = ctx.enter_context(tc.tile_pool(name="drp", bufs=2, space="DRAM"))

        # ---- persistent SBUF ----
        xkv_t = pers.tile([128, DC, L], f16, tag="xkv", name="xkv")
        xq_t = pers.tile([128, DC, L], f16, tag="xq", name="xq")
        kT = [pers.tile([128, L], bf16, tag=f"kT{p}", name=f"kT{p}") for p in range(NP)]
        qT = [pers.tile([128, L], bf16, tag=f"qT{p}", name=f"qT{p}") for p in range(NP)]
        # vnat: [lk-part, lk-chunk, head-blocks of (64 v | 1 ones)]
        vnat = pers.tile([128, LC, HC * VW], bf16, tag="vnat", name="vnat")
        Kw_t = pers.tile([128, DC, HA], f16, tag="Kw", name="Kw")
        Qw_t = pers.tile([128, DC, HA], f16, tag="Qw", name="Qw")
        Vw_t = pers.tile([128, DC, HA], f16, tag="Vw", name="Vw")
        ow = [pers.tile([128, D], bf16, tag=f"ow{c}", name=f"ow{c}") for c in range(NP)]
        warm = pers.tile([1, 16], f32, tag="warm", name="warm")
        # ones row on partition 64: stationary for the K=1 broadcast matmul
        ones65 = pers.tile([65, 64], f32, tag="ones65", name="ones65")
        nc.vector.memset(ones65[:], 1.0)

        # ---- PSUM (exactly 8 banks) ----
        sts = [psp.tile([128, 1024], f32, tag=f"sts{i}", name=f"sts{i}") for i in range(2)]
        pcs = [psp.tile([65, 512], f32, tag=f"pcs{h}", name=f"pcs{h}") for h in range(2)]
        util = [psp.tile([128, 512], f32, tag=f"util{i}", name=f"util{i}") for i in range(2)]

        # ones columns of vnat: element h*65+64 of each (c, h) block
        v0 = vnat[:]
        ones_ap = bass.AP(
            tensor=v0.tensor,
            offset=v0.offset + 64,
            ap=[list(v0.ap[0]), [HC * VW, LC], [VW, HC]],
        )
        nc.vector.memset(ones_ap, 1.0)
        # warm the exp table during idle lead-in
        nc.vector.memset(warm[:], 0.0)
        nc.scalar.activation(warm[:], warm[:], ExpF)

        # ---- DMAs: column-chunked x, ordered for earliest first scores ----
        def w_ap(w_e):
            return bass.AP(
                tensor=w_e.tensor,
                offset=w_e.offset,
                ap=[[HA, 128], [128 * HA, DC], [1, HA]],
            )

        def x_cc_ap(x_e, cc):
            # [128 part, DC d-chunks, 512 cols] of x^T column-chunk cc
            return bass.AP(
                tensor=x_e.tensor,
                offset=x_e.offset + cc * 512,
                ap=[[L, 128], [128 * L, DC], [1, 512]],
            )

        def w_slice_ap(w_e, lo, n):
            return bass.AP(
                tensor=w_e.tensor,
                offset=w_e.offset + lo,
                ap=[[HA, 128], [128 * HA, DC], [1, n]],
            )

        # pair-0 weight slices first: the first k/q units only need 128 of
        # the 512 weight columns, so the pipeline ignites ~6us earlier
        nc.sync.dma_start(out=Kw_t[:, :, 0:128], in_=w_slice_ap(Kw_e, 0, 128))
        nc.sync.dma_start(out=xkv_t[:, :, 0:512], in_=x_cc_ap(xkvT_e, 0))
        nc.sync.dma_start(out=Vw_t[:], in_=w_ap(Vw_e))
        nc.sync.dma_start(out=Qw_t[:, :, 0:128], in_=w_slice_ap(Qw_e, 0, 128))
        nc.sync.dma_start(out=xq_t[:, :, 0:512], in_=x_cc_ap(xqT_e, 0))
        for cc in range(1, 4):
            nc.sync.dma_start(
                out=xkv_t[:, :, cc * 512:(cc + 1) * 512], in_=x_cc_ap(xkvT_e, cc))
        nc.sync.dma_start(out=Kw_t[:, :, 128:512], in_=w_slice_ap(Kw_e, 128, 384))
        nc.sync.dma_start(out=Qw_t[:, :, 128:512], in_=w_slice_ap(Qw_e, 128, 384))
        for cc in range(1, 4):
            nc.sync.dma_start(
                out=xq_t[:, :, cc * 512:(cc + 1) * 512], in_=x_cc_ap(xqT_e, cc))
        for c in range(NP):
            nc.sync.dma_start(out=ow[c][:], in_=OwT_e[c * 128:(c + 1) * 128, :])

        # ---- weave units ----
        util_i = [0]

        def next_util():
            u = util[util_i[0]]
            util_i[0] ^= 1
            return u

        def k_unit(p, lqt):
            u = next_util()
            for d in range(DC):
                nc.tensor.matmul(
                    u[:], lhsT=Kw_t[:, d, p * 128:(p + 1) * 128],
                    rhs=xkv_t[:, d, lqt * 512:(lqt + 1) * 512],
                    start=(d == 0), stop=(d == DC - 1))
            nc.vector.tensor_copy(kT[p][:, lqt * 512:(lqt + 1) * 512], u[:])

        def q_unit(p, s):
            u = next_util()
            for d in range(DC):
                nc.tensor.matmul(
                    u[:], lhsT=Qw_t[:, d, p * 128:(p + 1) * 128],
                    rhs=xq_t[:, d, s * 512:(s + 1) * 512],
                    start=(d == 0), stop=(d == DC - 1))
            nc.vector.tensor_copy(qT[p][:, s * 512:(s + 1) * 512], u[:])

        def v_unit(c, half):
            # natural-layout v for lk-chunk c, heads 4*half..4*half+3
            u = next_util()
            for d in range(DC):
                nc.tensor.matmul(
                    u[:, 0:256], lhsT=xkv_t[:, d, c * 128:(c + 1) * 128],
                    rhs=Vw_t[:, d, half * 256:(half + 1) * 256],
                    start=(d == 0), stop=(d == DC - 1))
            vc = vnat[:, c, :]
            dst = bass.AP(
                tensor=vc.tensor,
                offset=vc.offset + 4 * half * VW,
                ap=[list(vc.ap[0]), [VW, 4], [1, 64]],
            )
            ua = u[:]
            src = bass.AP(
                tensor=ua.tensor,
                offset=ua.offset,
                ap=[list(ua.ap[0]), [64, 4], [1, 64]],
            )
            nc.vector.tensor_copy(dst, src)

        ctp = {}

        def opj_unit(s, lqs, dt):
            u = next_util()
            for p in range(NP):
                nc.tensor.matmul(
                    u[:], lhsT=ctp[(s, p)][:, lqs * 128:(lqs + 1) * 128],
                    rhs=ow[p][:, dt * 512:(dt + 1) * 512],
                    start=(p == 0), stop=(p == NP - 1))
            row = s * 512 + lqs * 128
            ost = ostp.tile([128, 512], f32, tag="ost", name=f"ost{s}_{lqs}_{dt}")
            nc.vector.tensor_copy(ost[:], u[:])
            nc.sync.dma_start(
                out=out_e[row:row + 128, dt * 512:(dt + 1) * 512], in_=ost[:])

        # ---- deadline-queue weave schedule ----
        # (deadline_iter, min_iter, unit); popped when deadline <= it+3, or
        # one filler per iteration once min_iter is reached.
        wq = []
        for lqt in range(1, 4):
            wq.append((4 * lqt - 1, 0, lambda lqt=lqt: k_unit(0, lqt)))
        for p in range(1, NP):
            for lqt in range(4):
                wq.append((16 * p + 4 * lqt - 1, 0, lambda p=p, lqt=lqt: k_unit(p, lqt)))
        for p in range(NP):
            for s in range(NS):
                if p == 0 and s == 0:
                    continue  # in lead
                wq.append((16 * (4 * s + p) - 1, 0, lambda p=p, s=s: q_unit(p, s)))
        for c in range(2, LC):
            wq.append((c, 0, lambda c=c: v_unit(c, 0)))
        for c in range(LC):
            wq.append((32 + c, 12, lambda c=c: v_unit(c, 1)))
        for s in range(NS - 1):
            # spread across the whole following strip window so the PE keeps
            # a filler unit in most iterations
            for j, (lqs, dt) in enumerate((a, b) for a in range(4) for b in range(2)):
                dl = 16 * (4 * s + 4) + 6 + 7 * j
                wq.append((dl, dl - 2,
                           lambda s=s, lqs=lqs, dt=dt: opj_unit(s, lqs, dt)))
        wq.sort(key=lambda t: t[0])

        # ---- normalize + pair-tile assembly per round ----
        def finalize(s, p, direct=False):
            # stage copies first: they feed the long denominator DMA chain
            stage = nrm.tile([65, 1024], f32, tag="stage", name=f"stage{s}_{p}")
            for h2 in range(2):
                nc.vector.tensor_copy(
                    stage[64:65, h2 * 512:(h2 + 1) * 512], pcs[h2][64:65, :])
            if direct:
                # last round: no next round needs pcs, multiply straight from
                # PSUM and skip the evacuation hop
                un = [pcs[0][0:64, :], pcs[1][0:64, :]]
            else:
                un = []
                for h2 in range(2):
                    ut = unp.tile([64, 512], bf16, tag="un", name=f"un{s}_{p}_{h2}")
                    nc.vector.tensor_copy(ut[:], pcs[h2][0:64, :])
                    un.append(ut[:])
            ct = ctpp.tile([128, 512], bf16, tag="ctp", name=f"ctp{s}_{p}")
            pbs = []
            if direct:
                # last round: latency-critical. Broadcast each denominator row
                # across partitions with a K=1 PE outer product (the PE is
                # idle and stays warm), then reciprocal on the spread rows —
                # no DMA round trips.
                for h2 in range(2):
                    nc.tensor.matmul(
                        util[h2][0:64, :], lhsT=ones65[64:65, :],
                        rhs=stage[64:65, h2 * 512:(h2 + 1) * 512],
                        start=True, stop=True)
                    pb = pbp.tile([64, 512], f32, tag="pbs", name=f"pbsd{h2}")
                    nc.vector.reciprocal_approx_fast(pb[:], util[h2][0:64, :])
                    pbs.append(pb)
            else:
                den = nrm.tile([2, 512], f32, tag="den", name=f"den{s}_{p}")
                nc.sync.dma_start(out=den[:], in_=stage[64:65, :])
                rec = nrm.tile([2, 512], f32, tag="rec", name=f"rec{s}_{p}")
                nc.vector.reciprocal_approx_fast(rec[:], den[:])
                # partition broadcast via DRAM bounce (0-stride partition
                # reads are only legal on DRAM APs)
                dr = drp.tile([2, 512], f32, tag="dr", name=f"dr{s}_{p}")
                nc.sync.dma_start(out=dr[:], in_=rec[:])
                for h2 in range(2):
                    pb = pbp.tile([64, 512], f32, tag="pbs", name=f"pbs{s}_{p}_{h2}")
                    rrow = dr[h2:h2 + 1, :]
                    rbc = bass.AP(
                        tensor=rrow.tensor, offset=rrow.offset,
                        ap=[[0, 64], [1, 512]])
                    nc.sync.dma_start(out=pb[:], in_=rbc)
                    pbs.append(pb)
            # odd head first: its SBUF->SBUF placement DMA overlaps h0's mult
            cto = ctop.tile([64, 512], bf16, tag="cto", name=f"cto{s}_{p}")
            nc.vector.tensor_mul(cto[:], un[1], pbs[1][:])
            nc.sync.dma_start(out=ct[64:128, :], in_=cto[:])
            nc.vector.tensor_mul(ct[0:64, :], un[0], pbs[0][:])
            ctp[(s, p)] = ct

        # ---- main pipeline ----
        sched = [(s, p, c) for s in range(NS) for p in range(NP) for c in range(LC)]
        NIT = len(sched)

        def emit_scores(it):
            s, p, c = sched[it]
            st = sts[it % 2]
            for h2 in range(2):
                base = 64 * h2
                nc.tensor.matmul(
                    st[:, h2 * 512:(h2 + 1) * 512],
                    lhsT=kT[p][base:base + 64, c * 128:(c + 1) * 128],
                    rhs=qT[p][base:base + 64, s * 512:(s + 1) * 512],
                    start=True, stop=True)

        # lead: minimal critical path to the first exp; v halves slot into
        # the DMA-arrival gaps so the PE never idles waiting for xq
        k_unit(0, 0)
        v_unit(0, 0)
        v_unit(1, 0)
        q_unit(0, 0)
        emit_scores(0)

        for it, (s, p, c) in enumerate(sched):
            if it + 1 < NIT:
                emit_scores(it + 1)
            et = etp.tile([128, 1024], bf16, tag="et", name=f"et{it}")
            nc.scalar.activation(et[:], sts[it % 2][:], ExpF)
            # weave before ctx: the PE chews projection/outproj work while
            # this iteration's exp finishes, instead of stalling at ctx
            emitted = 0
            while wq and wq[0][0] <= it + 3 and emitted < 2:
                wq.pop(0)[2]()
                emitted += 1
            if not emitted and wq and wq[0][1] <= it:
                wq.pop(0)[2]()
            for h2 in range(2):
                nc.tensor.matmul(
                    pcs[h2][:],
                    lhsT=vnat[:, c, (2 * p + h2) * VW:(2 * p + h2) * VW + VW],
                    rhs=et[:, h2 * 512:(h2 + 1) * 512],
                    start=(c == 0), stop=(c == LC - 1))
            if c == LC - 1:
                finalize(s, p, direct=(it == NIT - 1))

        while wq:
            wq.pop(0)[2]()

        # ---- tail: last strip's output projection ----
        for lqs in range(4):
            for dt in range(2):
                opj_unit(NS - 1, lqs, dt)

    nc.compile()
    return nc


_NC = None


def _get_nc():
    global _NC
    if _NC is None:
        _NC = build_graph()
    return _NC


# test harness can override, e.g. {"trace": True}
RUN_KWARGS: dict = {}
LAST_RESULTS = None


def make_in_maps(xq, xkv, Q, K, V, O):
    xq = np.asarray(xq, np.float32)
    xkv = np.asarray(xkv, np.float32)
    Q = np.asarray(Q, np.float32)
    K = np.asarray(K, np.float32)
    V = np.asarray(V, np.float32)
    O = np.asarray(O, np.float32)
    # cores 2b and 2b+1 share batch b's transposed activations; compute once
    xqT_c = [np.ascontiguousarray(xq[b].T).astype(np.float16) for b in range(B)]
    xkvT_c = [np.ascontiguousarray(xkv[b].T).astype(np.float16) for b in range(B)]
    in_maps = []
    for core in range(8):
        b, hg = divmod(core, 2)
        hs = slice(hg * HC, (hg + 1) * HC)
        in_maps.append({
            "xqT": xqT_c[b],
            "xkvT": xkvT_c[b],
            "Qw": np.ascontiguousarray(Q[:, hs, :].reshape(D, HA)).astype(np.float16),
            "Kw": np.ascontiguousarray(K[:, hs, :].reshape(D, HA)).astype(np.float16),
            "Vw": np.ascontiguousarray(V[:, hs, :].reshape(D, HA)).astype(np.float16),
            "OwT": np.ascontiguousarray(
                O[:, hs, :].reshape(D, HA).T).astype(ml_dtypes.bfloat16),
        })
    return in_maps


def kernel(xq, xkv, Q, K, V, O):
    global LAST_RESULTS
    nc = _get_nc()
    in_maps = make_in_maps(xq, xkv, Q, K, V, O)
    res = run_bass_kernel_spmd(nc, in_maps, core_ids=list(range(8)), **RUN_KWARGS)
    LAST_RESULTS = res
    outs = [np.asarray(res.results[c]["out"], np.float32) for c in range(8)]
    return np.stack([outs[2 * b] + outs[2 * b + 1] for b in range(B)], axis=0)
